# revision 50
# baseline (speedup 1.0000x reference)
"""Trainium2 Bass kernel for nn_BobaTransformerBlock (dense transformer block
with linear attention + poly-gelu MLP), data-parallel over batch on 8 cores.

Math (per sample, exact reassociation of the reference):
  h  = x * g1
  Gx = x^T x                                   [256,256]   (fp8 DoubleRow)
  per head h: KV_h = wk'_h Gx wv'_h^T ; MT_h = KV_h^T wq'_h   (wX' = wX * g1)
  P  = (SCALE/N) * M @ w_out^T ;  PI = P + I
  x2 = x @ PI + b_out                          (attention + residual)
  m  = x @ W1F + b1f          (W1F = PI @ w1g^T, b1f = b_out @ w1g^T + b1)
  nl = (0.0012 m + 0.1972) * m * m^2           (poly-gelu nonlinear part)
  y  = x @ WLF + nl @ w2^T + b2f
       (WLF = PI @ Wlin^T, Wlin = I + 0.5 w2 @ w1g,
        b2f = b2 + 0.5 w2 @ b1 + b_out @ Wlin^T)

Device layout is channel-major ("transposed"): activations [c, n] so biases
are per-partition. Host supplies x twice: p-major fp8 (Gram phase, DoubleRow
pairs) and transposed bf16 (everything else). The device writes y transposed
in bf16; the host transposes/casts back during unsharding.

Phase 2 runs on 1024-token macro tiles (512 at the pipeline fill/drain
edges); per tile the 8 MLP o-chunks each produce m in a 2-bank PSUM tile
that ACT drains in one [128,1024] op (one per-partition bias per o-chunk).
The gelu chain is split across ACT (Square), DVE (4x tensor_scalar /
2x tensor_tensor), and GpSimd to balance engine busy time, and the
previous tile's y output groups are interleaved between this tile's MLP1
chunks so the PE and ACT paces stay matched (the steady state has zero
PE idle). All phase-1.5 PSUM drains are split across ACT and DVE to
halve the serial-latency chain; y stores issue from the SP HWDGE ring.
"""

import sys

for _p in ("/opt/trn_rl_repo", "/opt/pypackages"):
    if _p not in sys.path:
        sys.path.insert(0, _p)

from contextlib import ExitStack

import numpy as np

import concourse.bass as bass
import concourse.mybir as mybir
import concourse.tile as tile
from concourse.bass_utils import run_bass_kernel_spmd

F32 = mybir.dt.float32
BF16 = mybir.dt.bfloat16
F8 = mybir.dt.float8e4
NP_BF16 = mybir.dt.np(BF16)
NP_F8 = mybir.dt.np(F8)
AF = mybir.ActivationFunctionType
ALU = mybir.AluOpType
MPM = mybir.MatmulPerfMode

B, N, C = 8, 8192, 256
H, D = 8, 64
INNER = H * D          # 512
MLP = 4 * C            # 1024
SCALE = 1.0 / np.sqrt(D)
S_ATTN = float(SCALE / N)
N_CORES = 8
NT = 1024              # phase-2 macro tile (tokens)
NJ = N // NT           # 8

# elementwise engine assignment per o-chunk (0..7)
S_POOL = (0, 4)        # tensor_scalar s on GpSimd for these chunks
T2_ACT = (1, 5)        # m^2 via ACT Square-from-PSUM
T2_POOL = (2, 6)       # m^2 via GpSimd tensor_tensor

_NC = None             # cached Bass program
LAST_RESULTS = None    # BassKernelResults of the most recent run (for test.py)


def _legalize_waits(nc, max_waits=1):
    """walrus's TPB codegen accepts at most one sync wait per instruction.
    Move excess waits onto preceding same-engine NOPs."""
    ctr = 0
    for f in nc.m.functions:
        for bb in f.blocks:
            insts = bb.instructions
            i = 0
            while i < len(insts):
                inst = insts[i]
                si = inst.sync_info
                waits = list(si.on_wait) if (si is not None and si.on_wait) else []
                if len(waits) > max_waits:
                    keep = waits[-max_waits:]
                    extra = waits[:-max_waits]
                    pos = i
                    while extra:
                        chunk, extra = extra[:max_waits], extra[max_waits:]
                        nop = mybir.InstNoOp(
                            name=f"I-waitsplit-{ctr}",
                            engine=inst.engine,
                            ins=[],
                            outs=[],
                            sync_info=mybir.SyncInfo(on_wait=chunk, on_update=[]),
                        )
                        ctr += 1
                        insts.insert(pos, nop)
                        pos += 1
                        i += 1
                    inst.sync_info = mybir.SyncInfo(
                        on_wait=keep,
                        on_update=list(si.on_update) if si.on_update else [],
                    )
                i += 1
    return ctr


def _build_program(reps=1):
    nc = bass.Bass(trn_type="TRN2")

    xq_d = nc.declare_dram_parameter("xq", [128, 4, 16, 256], F8, isOutput=False)
    xt_d = nc.declare_dram_parameter("xt", [C, N], BF16, isOutput=False)
    wkv_d = nc.declare_dram_parameter("wkv", [128, 2, 2 * INNER], BF16, isOutput=False)
    wq_d = nc.declare_dram_parameter("wq", [64, H, C], BF16, isOutput=False)
    wo_d = nc.declare_dram_parameter("wo", [128, 4, C], BF16, isOutput=False)
    w1t_d = nc.declare_dram_parameter("w1t", [128, 2, MLP], BF16, isOutput=False)
    w2t_d = nc.declare_dram_parameter("w2t", [128, 8, C], BF16, isOutput=False)
    wlin_d = nc.declare_dram_parameter("wlin", [128, 2, C], BF16, isOutput=False)
    ident_d = nc.declare_dram_parameter("ident", [128, 2, C], BF16, isOutput=False)
    bias_d = nc.declare_dram_parameter("bias", [128, 12], F32, isOutput=False)
    yt_d = nc.declare_dram_parameter("yt", [2, 128, N], BF16, isOutput=True)

    with tile.TileContext(nc) as tc, ExitStack() as ctx:
        const = ctx.enter_context(tc.tile_pool(name="const", bufs=1))
        wkv = const.tile([128, 2, 2 * INNER], BF16, name="wkv", tag="wkv")
        wq = const.tile([64, H, C], BF16, name="wq", tag="wq")
        wo = const.tile([128, 4, C], BF16, name="wo", tag="wo")
        w1t = const.tile([128, 2, MLP], BF16, name="w1t", tag="w1t")
        w2t = const.tile([128, 8, C], BF16, name="w2t", tag="w2t")
        wlin = const.tile([128, 2, C], BF16, name="wlin", tag="wlin")
        ident = const.tile([128, 2, C], BF16, name="ident", tag="ident")
        bias = const.tile([128, 12], F32, name="bias", tag="bias")
        G_sb = const.tile([128, 2, C], BF16, name="G", tag="G")
        MT_sb = const.tile([128, 4, C], BF16, name="MT", tag="MT")
        xt_res = const.tile([128, 2, N], BF16, name="xt_res", tag="xt_res")
        PIT = const.tile([128, 2, C], BF16, name="PIT", tag="PIT")
        W1F = const.tile([128, 2, MLP], BF16, name="W1F", tag="W1F")
        WLF = const.tile([128, 2, C], BF16, name="WLF", tag="WLF")

        for _rep in range(reps):

            # ---------------- Phase 1: Gram matrix Gx = x^T x (fp8 DR) ----------------
            with tc.tile_pool(name="xp", bufs=4) as xp, \
                 tc.tile_pool(name="gps", bufs=1, space="PSUM") as gps:
                g_ps = [gps.tile([128, C], F32, name=f"g{k}", tag=f"g{k}") for k in range(2)]
                for tb in range(4):
                    if tb == 0:
                        # first chunk in two independent tiles so the Gram can
                        # start as soon as the first half-DMA lands
                        halves = []
                        for hh in range(2):
                            xh = xp.tile([128, 8, 256], F8, name="xh", tag=f"xh{hh}")
                            nc.sync.dma_start(out=xh[:],
                                              in_=xq_d[:, 0, hh * 8:(hh + 1) * 8, :])
                            halves.append(xh)
                        def xsl(u, ksl):
                            return halves[u // 4][:, 2 * (u % 4):2 * (u % 4) + 2, ksl]
                    else:
                        x_t = xp.tile([128, 16, 256], F8, name="x", tag="x")
                        nc.sync.dma_start(out=x_t[:], in_=xq_d[:, tb, :, :])
                        def xsl(u, ksl):
                            return x_t[:, 2 * u:2 * u + 2, ksl]
                    for u in range(8):
                        for k in range(2):
                            nc.tensor.matmul(
                                g_ps[k][:],
                                lhsT=xsl(u, slice(k * 128, (k + 1) * 128)),
                                rhs=xsl(u, slice(0, 256)),
                                start=(u == 0 and tb == 0),
                                stop=(u == 7 and tb == 3),
                                perf_mode=MPM.DoubleRow,
                            )
                # copies split across engines so each atall matmul's two G
                # chunks (k is its contraction side) land in parallel
                nc.scalar.activation(out=G_sb[:, 0, :], in_=g_ps[0][:], func=AF.Copy)
                nc.vector.tensor_copy(out=G_sb[:, 1, :], in_=g_ps[1][:])

            # Ordered input stream on the SP HWDGE ring (just-in-time): Gram xq
            # chunks were emitted above; phase-1.5 weights, then the phase-2
            # weights, then xt in per-macro-tile chunks.
            nc.sync.dma_start(out=wkv[:, :, 0:INNER], in_=wkv_d[:, :, 0:INNER])
            nc.sync.dma_start(out=wkv[:, :, INNER:2 * INNER],
                              in_=wkv_d[:, :, INNER:2 * INNER])
            for sb, dr in ((wq, wq_d), (wo, wo_d), (ident, ident_d),
                           (wlin, wlin_d), (w1t, w1t_d)):
                nc.sync.dma_start(out=sb[:], in_=dr[:])
            for k in range(2):
                nc.sync.dma_start(out=xt_res[:, k, 0:NT],
                                  in_=xt_d[k * 128:(k + 1) * 128, 0:NT])
            for sb, dr in ((bias, bias_d), (w2t, w2t_d)):
                nc.sync.dma_start(out=sb[:], in_=dr[:])
            for J in range(1, NJ):
                for k in range(2):
                    nc.sync.dma_start(
                        out=xt_res[:, k, J * NT:(J + 1) * NT],
                        in_=xt_d[k * 128:(k + 1) * 128, J * NT:(J + 1) * NT])

            # ---------------- Phase 1.5: per-head KV path -> PI -> folds ----------------
            with tc.tile_pool(name="hsb", bufs=6) as hsb, \
                 tc.tile_pool(name="hps", bufs=6, space="PSUM") as hps, \
                 tc.tile_pool(name="pps", bufs=1, space="PSUM") as pps:
                # ATall = Gx @ wk'^T for all heads at once (Gx is symmetric, so
                # no transpose of the intermediate is ever needed)
                atall = hsb.tile([128, 2, INNER], BF16, name="atall", tag="atall")
                for cc in range(2):
                    at_ps = hps.tile([128, INNER], F32, name="hps", tag="hps")
                    for k2 in range(2):
                        nc.tensor.matmul(
                            at_ps[:],
                            lhsT=G_sb[:, k2, cc * 128:(cc + 1) * 128],
                            rhs=wkv[:, k2, 0:INNER],
                            start=(k2 == 0), stop=(k2 == 1),
                        )
                    if cc == 0:
                        nc.scalar.activation(out=atall[:, cc, :], in_=at_ps[:],
                                             func=AF.Copy)
                    else:
                        nc.vector.tensor_copy(out=atall[:, cc, :], in_=at_ps[:])

                # all 8 heads batched: KV_h side by side in the free dim so the
                # tiny per-head matmuls stream back-to-back with batched copies
                kv_sb = hsb.tile([64, 8, 64], BF16, name="kv", tag="kv")
                for half in range(2):
                    kvp_ps = hps.tile([64, 256], F32, name="hps", tag="hps")
                    for i in range(4):
                        h = 4 * half + i
                        for kk in range(2):
                            nc.tensor.matmul(
                                kvp_ps[:, i * 64:(i + 1) * 64],
                                lhsT=atall[:, kk, h * 64:(h + 1) * 64],
                                rhs=wkv[:, kk, INNER + h * 64:INNER + (h + 1) * 64],
                                start=(kk == 0), stop=(kk == 1),
                            )
                    if half == 0:
                        nc.scalar.activation(out=kv_sb[:, 0:4, :], in_=kvp_ps[:],
                                             func=AF.Copy)
                    else:
                        nc.vector.tensor_copy(out=kv_sb[:, 4:8, :], in_=kvp_ps[:])
                for hp in range(4):
                    mtp_ps = hps.tile([128, C], F32, name="hps", tag="hps")
                    for i in range(2):
                        h = 2 * hp + i
                        nc.tensor.matmul(
                            mtp_ps[i * 64:(i + 1) * 64, :],
                            lhsT=kv_sb[:, h, :],
                            rhs=wq[:, h, :], start=True, stop=True)
                    if hp % 2 == 0:
                        nc.scalar.activation(out=MT_sb[:, hp, :], in_=mtp_ps[:],
                                             func=AF.Copy)
                    else:
                        nc.vector.tensor_copy(out=MT_sb[:, hp, :], in_=mtp_ps[:])

                # P^T = w_out @ M directly (lhsT = w_out^T = wo, rhs = M^T = MT_sb),
                # then PIT = S_ATTN * P^T + I -- no PE transpose round-trip needed.
                for cc in range(2):
                    p_ps = pps.tile([128, C], F32, name=f"p{cc}", tag=f"p{cc}")
                    for kk in range(4):
                        nc.tensor.matmul(
                            p_ps[:],
                            lhsT=wo[:, kk, cc * 128:(cc + 1) * 128],
                            rhs=MT_sb[:, kk, :],
                            start=(kk == 0), stop=(kk == 3),
                        )
                    for hb in range(2):
                        nc.vector.scalar_tensor_tensor(
                            out=PIT[:, cc, hb * 128:(hb + 1) * 128],
                            in0=p_ps[:, hb * 128:(hb + 1) * 128], scalar=S_ATTN,
                            in1=ident[:, cc, hb * 128:(hb + 1) * 128],
                            op0=ALU.mult, op1=ALU.add,
                        )
                # W1F = PI @ w1g^T and WLF = PI @ Wlin^T: fold the attention
                # apply into the MLP/output weights so x2 is never materialized.
                for cb in range(2):
                    for oh in range(2):
                        wf_ps = hps.tile([128, 512], F32, name="wf", tag="hps")
                        for k2 in range(2):
                            nc.tensor.matmul(
                                wf_ps[:],
                                lhsT=PIT[:, k2, cb * 128:(cb + 1) * 128],
                                rhs=w1t[:, k2, oh * 512:(oh + 1) * 512],
                                start=(k2 == 0), stop=(k2 == 1),
                            )
                        if cb == 0:
                            nc.scalar.activation(
                                out=W1F[:, cb, oh * 512:(oh + 1) * 512],
                                in_=wf_ps[:], func=AF.Copy)
                        else:
                            nc.vector.tensor_copy(
                                out=W1F[:, cb, oh * 512:(oh + 1) * 512],
                                in_=wf_ps[:])
                    wl_ps = hps.tile([128, C], F32, name="wl", tag="hps")
                    for k2 in range(2):
                        nc.tensor.matmul(
                            wl_ps[:],
                            lhsT=PIT[:, k2, cb * 128:(cb + 1) * 128],
                            rhs=wlin[:, k2, :],
                            start=(k2 == 0), stop=(k2 == 1),
                        )
                    if cb == 0:
                        nc.scalar.activation(out=WLF[:, cb, :], in_=wl_ps[:],
                                             func=AF.Copy)
                    else:
                        nc.vector.tensor_copy(out=WLF[:, cb, :], in_=wl_ps[:])

            # ---------------- Phase 2: streamed MLP (attention pre-folded) ----------------
            with tc.tile_pool(name="gel", bufs=4) as gel, \
                 tc.tile_pool(name="nlp", bufs=2) as nlp, \
                 tc.tile_pool(name="yp", bufs=3) as yp, \
                 tc.tile_pool(name="mps", bufs=3, space="PSUM") as mps, \
                 tc.tile_pool(name="yps", bufs=2, space="PSUM") as yps:
                def emit_y_group(base, nl, cc, nh):
                    # y = x @ WLF + nl @ w2^T + b2f  (one [128,512] output group;
                    # groups of the previous tile are interleaved between this
                    # tile's MLP1 chunks so the PE and ACT paces stay matched)
                    sl = slice(base + nh * 512, base + (nh + 1) * 512)
                    y_ps = yps.tile([128, 512], F32, name="y", tag="y")
                    for k in range(2):
                        nc.tensor.matmul(
                            y_ps[:],
                            lhsT=WLF[:, k, cc * 128:(cc + 1) * 128],
                            rhs=xt_res[:, k, sl],
                            start=(k == 0), stop=False,
                        )
                    for kk in range(8):
                        nc.tensor.matmul(
                            y_ps[:],
                            lhsT=w2t[:, kk, cc * 128:(cc + 1) * 128],
                            rhs=nl[:, kk, nh, :],
                            start=False, stop=(kk == 7),
                        )
                    y_sb = yp.tile([128, 512], BF16, name="ysb", tag="ysb")
                    nc.scalar.activation(out=y_sb[:], in_=y_ps[:], func=AF.Identity,
                                         bias=bias[:, 10 + cc:11 + cc])
                    nc.sync.dma_start(out=yt_d[cc, :, sl], in_=y_sb[:])

                # tiles: (token base, 512-token subtile count); the first and
                # last macro tiles are split so the pipeline fills and drains
                # with less PE idle time
                TILES = ([(0, 1)] + [(512 + j * NT, 2) for j in range(NJ - 2)]
                         + [(6656, 1), (7168, 1), (7680, 1)])
                pending = None      # (base, nhc, nl, [groups left to emit])
                for ti, (base, nhc) in enumerate(TILES):
                    # MLP hidden + poly-gelu nonlinear part, one [128,nhc*512]
                    # chunk per MLP o-block (single per-partition bias each):
                    #   m  = x @ W1F + b1f      (PSUM, drained by ACT)
                    #   s  = 0.0012 m + 0.1972
                    #   t1 = m * s ; t2 = m^2 ; nl = t1 * t2
                    # j0 keeps Square off ACT (no y copies to hide it behind);
                    # j1 interleaves the previous tile's groups later because
                    # DVE is still catching up on tile 0's chains.
                    t2_act = () if ti == 0 else T2_ACT
                    t2_pool = (1, 2, 5, 6) if ti == 0 else T2_POOL
                    ilv = (2, 4, 6)
                    last = ti == len(TILES) - 1
                    nl = nlp.tile([128, 8, 2, 512], BF16, name="nl", tag="nl")
                    for o in range(8):
                        bcol = bias[:, 2 + o:3 + o]
                        m_ps = mps.tile([128, 2, 512], F32, name="m", tag="m")
                        for nh in range(nhc):
                            sl = slice(base + nh * 512, base + (nh + 1) * 512)
                            for k in range(2):
                                nc.tensor.matmul(
                                    m_ps[:, nh, :],
                                    lhsT=W1F[:, k, o * 128:(o + 1) * 128],
                                    rhs=xt_res[:, k, sl],
                                    start=(k == 0), stop=(k == 1),
                                )
                        ma = gel.tile([128, 2, 512], BF16, name="ma", tag="ma")
                        nc.scalar.activation(out=ma[:, 0:nhc, :], in_=m_ps[:, 0:nhc, :],
                                             func=AF.Identity, bias=bcol)
                        s = gel.tile([128, 2, 512], BF16, name="s", tag="s")
                        s_eng = nc.gpsimd if o in S_POOL else nc.vector
                        s_eng.tensor_scalar(out=s[:, 0:nhc, :], in0=ma[:, 0:nhc, :],
                                            scalar1=0.0012, scalar2=0.1972,
                                            op0=ALU.mult, op1=ALU.add)
                        t1 = gel.tile([128, 2, 512], BF16, name="t1", tag="t1")
                        nc.vector.tensor_tensor(out=t1[:, 0:nhc, :], in0=ma[:, 0:nhc, :],
                                                in1=s[:, 0:nhc, :], op=ALU.mult)
                        t2 = gel.tile([128, 2, 512], BF16, name="t2", tag="t2")
                        if o in t2_act:
                            nc.scalar.activation(out=t2[:, 0:nhc, :], in_=ma[:, 0:nhc, :],
                                                 func=AF.Square)
                        elif o in t2_pool:
                            nc.gpsimd.tensor_tensor(out=t2[:, 0:nhc, :], in0=ma[:, 0:nhc, :],
                                                    in1=ma[:, 0:nhc, :], op=ALU.mult)
                        else:
                            nc.vector.tensor_tensor(out=t2[:, 0:nhc, :], in0=ma[:, 0:nhc, :],
                                                    in1=ma[:, 0:nhc, :], op=ALU.mult)
                        nc.vector.tensor_tensor(out=nl[:, o, 0:nhc, :], in0=t1[:, 0:nhc, :],
                                                in1=t2[:, 0:nhc, :], op=ALU.mult)
                        if pending is not None and pending[3] and o in ilv:
                            g = pending[3].pop(0)
                            emit_y_group(pending[0], pending[2], *g)
                    if last:
                        # final 512 tokens: both groups emitted back-to-back,
                        # with the PSUM drains and stores on parallel engine
                        # paths (ACT+SP for cc=0, DVE+ACT-DGE for cc=1)
                        sl = slice(base, base + 512)
                        for cc in range(2):
                            y_ps = yps.tile([128, 512], F32, name="y", tag="y")
                            for k in range(2):
                                nc.tensor.matmul(
                                    y_ps[:],
                                    lhsT=WLF[:, k, cc * 128:(cc + 1) * 128],
                                    rhs=xt_res[:, k, sl],
                                    start=(k == 0), stop=False,
                                )
                            for kk in range(8):
                                nc.tensor.matmul(
                                    y_ps[:],
                                    lhsT=w2t[:, kk, cc * 128:(cc + 1) * 128],
                                    rhs=nl[:, kk, 0, :],
                                    start=False, stop=(kk == 7),
                                )
                            if cc == 0:
                                y_sb0 = yp.tile([128, 512], BF16, name="ysb", tag="ysb")
                                nc.scalar.activation(out=y_sb0[:], in_=y_ps[:],
                                                     func=AF.Identity,
                                                     bias=bias[:, 10:11])
                                nc.sync.dma_start(out=yt_d[0, :, sl], in_=y_sb0[:])
                            else:
                                y_sb1 = yp.tile([128, 512], BF16, name="ysb2", tag="ysb2")
                                nc.vector.tensor_scalar(out=y_sb1[:], in0=y_ps[:],
                                                        scalar1=bias[:, 11:12],
                                                        scalar2=None, op0=ALU.add)
                                nc.scalar.dma_start(out=yt_d[1, :, sl], in_=y_sb1[:])
                        break
                    if pending is not None:
                        for g in pending[3]:
                            emit_y_group(pending[0], pending[2], *g)
                    groups = [(cc, nh) for nh in range(nhc) for cc in (0, 1)]
                    pending = (base, nhc, nl, groups)

    _legalize_waits(nc, 1)
    return nc


def _get_program(reps=1):
    global _NC
    if reps != 1:
        return _build_program(reps)
    if _NC is None:
        _NC = _build_program()
    return _NC


def _prep_maps(x, gamma1, w_qkv, w_out, b_out, gamma2, w1, b1, w2, b2):
    f8 = np.float64
    x = np.asarray(x, np.float32)
    g1 = np.asarray(gamma1, f8)
    g2 = np.asarray(gamma2, f8)
    w_qkv = np.asarray(w_qkv, f8)
    w_out = np.asarray(w_out, f8)
    b_out = np.asarray(b_out, f8)
    w1 = np.asarray(w1, f8)
    b1 = np.asarray(b1, f8)
    w2 = np.asarray(w2, f8)
    b2 = np.asarray(b2, f8)

    wq = w_qkv[0:INNER] * g1[None, :]
    wk = w_qkv[INNER:2 * INNER] * g1[None, :]
    wv = w_qkv[2 * INNER:3 * INNER] * g1[None, :]
    w1g = w1 * g2[None, :]
    wlin_m = np.eye(C) + 0.5 * (w2 @ w1g)       # [c', c]
    b2v = b2 + 0.5 * (w2 @ b1)

    def pk(a, kdim):  # [kdim*128, F] -> [128, kdim, F]
        return np.ascontiguousarray(
            a.reshape(kdim, 128, a.shape[-1]).transpose(1, 0, 2)).astype(NP_BF16)

    wkvT = np.concatenate([wk.T, wv.T], axis=1)             # [256, 1024]
    wkv_h = pk(wkvT, 2)
    wq_h = np.ascontiguousarray(
        wq.reshape(H, 64, C).transpose(1, 0, 2)).astype(NP_BF16)
    wo_h = pk(w_out.T.copy(), 4)                            # [512,256]->[128,4,256]
    w1t_h = pk(w1g.T.copy(), 2)                             # [256,1024]->[128,2,1024]
    w2t_h = np.ascontiguousarray(
        w2.T.reshape(8, 128, C).transpose(1, 0, 2)).astype(NP_BF16)
    wlin_h = pk(wlin_m.T.copy(), 2)                         # [256,256]->[128,2,256]
    ident_h = pk(np.eye(C), 2)
    b1f = b_out @ w1g.T + b1                                # [1024]
    b2f = b2v + b_out @ wlin_m.T                            # [256]
    bias_h = np.concatenate([
        b_out.reshape(2, 128).T, b1f.reshape(8, 128).T, b2f.reshape(2, 128).T,
    ], axis=1).astype(np.float32)                           # [128, 12]
    shared = dict(wkv=wkv_h, wq=wq_h, wo=wo_h, w1t=w1t_h, w2t=w2t_h,
                  wlin=wlin_h, ident=ident_h, bias=bias_h)

    in_maps = []
    for b in range(B):
        xb = np.ascontiguousarray(x[b])
        # p-major fp8 layout for the DoubleRow Gram:
        # n = tb*2048 + u*256 + i*128 + p  ->  [p, tb, u*2+i, c]
        xq = np.ascontiguousarray(
            xb.reshape(4, 8, 2, 128, C).transpose(3, 0, 1, 2, 4)
            .reshape(128, 4, 16, C)).astype(NP_F8)
        in_maps.append(dict(xq=xq, xt=np.ascontiguousarray(xb.T).astype(NP_BF16),
                            **shared))
    return in_maps


def kernel(**inputs):
    global LAST_RESULTS
    nc = _get_program()
    in_maps = _prep_maps(**inputs)
    res = run_bass_kernel_spmd(nc, in_maps, list(range(N_CORES)))
    LAST_RESULTS = res
    out = np.empty((B, N, C), np.float32)
    for b in range(B):
        yt = np.asarray(res.results[b]["yt"]).astype(np.float32)   # [2, 128, N]
        out[b] = yt.reshape(C, N).T
    return out


# revision 59
# speedup vs baseline: 1.0051x; 1.0051x over previous
"""Trainium2 Bass kernel for nn_BobaTransformerBlock (dense transformer block
with linear attention + poly-gelu MLP), data-parallel over batch on 8 cores.

Math (per sample, exact reassociation of the reference):
  h  = x * g1
  Gx = x^T x                                   [256,256]   (fp8 DoubleRow)
  per head h: KV_h = wk'_h Gx wv'_h^T ; MT_h = KV_h^T wq'_h   (wX' = wX * g1)
  P  = (SCALE/N) * M @ w_out^T ;  PI = P + I
  x2 = x @ PI + b_out                          (attention + residual)
  m  = x @ W1F + b1f          (W1F = PI @ w1g^T, b1f = b_out @ w1g^T + b1)
  nl = (0.0012 m + 0.1972) * m * m^2           (poly-gelu nonlinear part)
  y  = x @ WLF + nl @ w2^T + b2f
       (WLF = PI @ Wlin^T, Wlin = I + 0.5 w2 @ w1g,
        b2f = b2 + 0.5 w2 @ b1 + b_out @ Wlin^T)

Device layout is channel-major ("transposed"): activations [c, n] so biases
are per-partition. Host supplies x twice: p-major fp8 (Gram phase, DoubleRow
pairs) and transposed bf16 (everything else). The device writes y transposed
in bf16; the host transposes/casts back during unsharding.

Phase 2 runs on 1024-token macro tiles (512 at the pipeline fill/drain
edges); per tile the 8 MLP o-chunks each produce m in a 2-bank PSUM tile
that ACT drains in one [128,1024] op (one per-partition bias per o-chunk).
The gelu chain is split across ACT (Square), DVE (4x tensor_scalar /
2x tensor_tensor), and GpSimd to balance engine busy time, and the
previous tile's y output groups are interleaved between this tile's MLP1
chunks so the PE and ACT paces stay matched (the steady state has zero
PE idle). All phase-1.5 PSUM drains are split across ACT and DVE to
halve the serial-latency chain; y stores issue from the SP HWDGE ring.
"""

import sys

for _p in ("/opt/trn_rl_repo", "/opt/pypackages"):
    if _p not in sys.path:
        sys.path.insert(0, _p)

from contextlib import ExitStack

import numpy as np

import concourse.bass as bass
import concourse.mybir as mybir
import concourse.tile as tile
from concourse.bass_utils import run_bass_kernel_spmd

F32 = mybir.dt.float32
BF16 = mybir.dt.bfloat16
F8 = mybir.dt.float8e4
NP_BF16 = mybir.dt.np(BF16)
NP_F8 = mybir.dt.np(F8)
AF = mybir.ActivationFunctionType
ALU = mybir.AluOpType
MPM = mybir.MatmulPerfMode

B, N, C = 8, 8192, 256
H, D = 8, 64
INNER = H * D          # 512
MLP = 4 * C            # 1024
SCALE = 1.0 / np.sqrt(D)
S_ATTN = float(SCALE / N)
N_CORES = 8
NT = 1024              # phase-2 macro tile (tokens)
NJ = N // NT           # 8

# elementwise engine assignment per o-chunk (0..7)
S_POOL = (0, 4)        # tensor_scalar s on GpSimd for these chunks
T2_ACT = (1, 5)        # m^2 via ACT Square-from-PSUM
T2_POOL = (2, 6)       # m^2 via GpSimd tensor_tensor

_NC = None             # cached Bass program
LAST_RESULTS = None    # BassKernelResults of the most recent run (for test.py)


def _legalize_waits(nc, max_waits=1):
    """walrus's TPB codegen accepts at most one sync wait per instruction.
    Move excess waits onto preceding same-engine NOPs."""
    ctr = 0
    for f in nc.m.functions:
        for bb in f.blocks:
            insts = bb.instructions
            i = 0
            while i < len(insts):
                inst = insts[i]
                si = inst.sync_info
                waits = list(si.on_wait) if (si is not None and si.on_wait) else []
                if len(waits) > max_waits:
                    keep = waits[-max_waits:]
                    extra = waits[:-max_waits]
                    pos = i
                    while extra:
                        chunk, extra = extra[:max_waits], extra[max_waits:]
                        nop = mybir.InstNoOp(
                            name=f"I-waitsplit-{ctr}",
                            engine=inst.engine,
                            ins=[],
                            outs=[],
                            sync_info=mybir.SyncInfo(on_wait=chunk, on_update=[]),
                        )
                        ctr += 1
                        insts.insert(pos, nop)
                        pos += 1
                        i += 1
                    inst.sync_info = mybir.SyncInfo(
                        on_wait=keep,
                        on_update=list(si.on_update) if si.on_update else [],
                    )
                i += 1
    return ctr


def _build_program(reps=1):
    nc = bass.Bass(trn_type="TRN2")

    xq_d = nc.declare_dram_parameter("xq", [128, 4, 16, 256], F8, isOutput=False)
    xt_d = nc.declare_dram_parameter("xt", [C, N], BF16, isOutput=False)
    wkv_d = nc.declare_dram_parameter("wkv", [128, 2, 2 * INNER], BF16, isOutput=False)
    wq_d = nc.declare_dram_parameter("wq", [64, H, C], BF16, isOutput=False)
    wo_d = nc.declare_dram_parameter("wo", [128, 4, C], BF16, isOutput=False)
    w1t_d = nc.declare_dram_parameter("w1t", [128, 2, MLP], BF16, isOutput=False)
    w2t_d = nc.declare_dram_parameter("w2t", [128, 8, C], BF16, isOutput=False)
    wlin_d = nc.declare_dram_parameter("wlin", [128, 2, C], BF16, isOutput=False)
    ident_d = nc.declare_dram_parameter("ident", [128, 2, C], BF16, isOutput=False)
    bias_d = nc.declare_dram_parameter("bias", [128, 12], F32, isOutput=False)
    yt_d = nc.declare_dram_parameter("yt", [2, 128, N], BF16, isOutput=True)

    with tile.TileContext(nc) as tc, ExitStack() as ctx:
        const = ctx.enter_context(tc.tile_pool(name="const", bufs=1))
        wkv = const.tile([128, 2, 2 * INNER], BF16, name="wkv", tag="wkv")
        wq = const.tile([64, H, C], BF16, name="wq", tag="wq")
        wo = const.tile([128, 4, C], BF16, name="wo", tag="wo")
        w1t = const.tile([128, 2, MLP], BF16, name="w1t", tag="w1t")
        w2t = const.tile([128, 8, C], BF16, name="w2t", tag="w2t")
        wlin = const.tile([128, 2, C], BF16, name="wlin", tag="wlin")
        ident = const.tile([128, 2, C], BF16, name="ident", tag="ident")
        bias = const.tile([128, 12], F32, name="bias", tag="bias")
        G_sb = const.tile([128, 2, C], BF16, name="G", tag="G")
        MT_sb = const.tile([128, 4, C], BF16, name="MT", tag="MT")
        xt_res = const.tile([128, 2, N], BF16, name="xt_res", tag="xt_res")
        PIT = const.tile([128, 2, C], BF16, name="PIT", tag="PIT")
        W1F = const.tile([128, 2, MLP], BF16, name="W1F", tag="W1F")
        WLF = const.tile([128, 2, C], BF16, name="WLF", tag="WLF")

        for _rep in range(reps):

            # ---------------- Phase 1: Gram matrix Gx = x^T x (fp8 DR) ----------------
            with tc.tile_pool(name="xp", bufs=4) as xp, \
                 tc.tile_pool(name="gps", bufs=1, space="PSUM") as gps:
                g_ps = [gps.tile([128, C], F32, name=f"g{k}", tag=f"g{k}") for k in range(2)]
                for tb in range(4):
                    if tb == 0:
                        # first chunk in two independent tiles so the Gram can
                        # start as soon as the first half-DMA lands
                        halves = []
                        for hh in range(2):
                            xh = xp.tile([128, 8, 256], F8, name="xh", tag=f"xh{hh}")
                            nc.sync.dma_start(out=xh[:],
                                              in_=xq_d[:, 0, hh * 8:(hh + 1) * 8, :])
                            halves.append(xh)
                        def xsl(u, ksl):
                            return halves[u // 4][:, 2 * (u % 4):2 * (u % 4) + 2, ksl]
                    else:
                        x_t = xp.tile([128, 16, 256], F8, name="x", tag="x")
                        nc.sync.dma_start(out=x_t[:], in_=xq_d[:, tb, :, :])
                        def xsl(u, ksl):
                            return x_t[:, 2 * u:2 * u + 2, ksl]
                    for u in range(8):
                        for k in range(2):
                            nc.tensor.matmul(
                                g_ps[k][:],
                                lhsT=xsl(u, slice(k * 128, (k + 1) * 128)),
                                rhs=xsl(u, slice(0, 256)),
                                start=(u == 0 and tb == 0),
                                stop=(u == 7 and tb == 3),
                                perf_mode=MPM.DoubleRow,
                            )
                # copies split across engines so each atall matmul's two G
                # chunks (k is its contraction side) land in parallel
                nc.scalar.activation(out=G_sb[:, 0, :], in_=g_ps[0][:], func=AF.Copy)
                nc.vector.tensor_copy(out=G_sb[:, 1, :], in_=g_ps[1][:])

            # Ordered input stream on the SP HWDGE ring (just-in-time): Gram xq
            # chunks were emitted above; phase-1.5 weights, then the phase-2
            # weights, then xt in per-macro-tile chunks.
            nc.sync.dma_start(out=wkv[:, :, 0:INNER], in_=wkv_d[:, :, 0:INNER])
            nc.sync.dma_start(out=wkv[:, :, INNER:2 * INNER],
                              in_=wkv_d[:, :, INNER:2 * INNER])
            for sb, dr in ((wq, wq_d), (wo, wo_d), (ident, ident_d),
                           (wlin, wlin_d), (w1t, w1t_d)):
                nc.sync.dma_start(out=sb[:], in_=dr[:])
            for k in range(2):
                nc.sync.dma_start(out=xt_res[:, k, 0:NT],
                                  in_=xt_d[k * 128:(k + 1) * 128, 0:NT])
            for sb, dr in ((bias, bias_d), (w2t, w2t_d)):
                nc.sync.dma_start(out=sb[:], in_=dr[:])
            for J in range(1, NJ):
                for k in range(2):
                    nc.sync.dma_start(
                        out=xt_res[:, k, J * NT:(J + 1) * NT],
                        in_=xt_d[k * 128:(k + 1) * 128, J * NT:(J + 1) * NT])

            # ---------------- Phase 1.5: per-head KV path -> PI -> folds ----------------
            with tc.tile_pool(name="hsb", bufs=6) as hsb, \
                 tc.tile_pool(name="hps", bufs=6, space="PSUM") as hps, \
                 tc.tile_pool(name="pps", bufs=1, space="PSUM") as pps:
                # ATall = Gx @ wk'^T for all heads at once (Gx is symmetric, so
                # no transpose of the intermediate is ever needed)
                atall = hsb.tile([128, 2, INNER], BF16, name="atall", tag="atall")
                for cc in range(2):
                    at_ps = hps.tile([128, INNER], F32, name="hps", tag="hps")
                    for k2 in range(2):
                        nc.tensor.matmul(
                            at_ps[:],
                            lhsT=G_sb[:, k2, cc * 128:(cc + 1) * 128],
                            rhs=wkv[:, k2, 0:INNER],
                            start=(k2 == 0), stop=(k2 == 1),
                        )
                    if cc == 0:
                        nc.scalar.activation(out=atall[:, cc, :], in_=at_ps[:],
                                             func=AF.Copy)
                    else:
                        nc.vector.tensor_copy(out=atall[:, cc, :], in_=at_ps[:])

                # all 8 heads batched: KV_h side by side in the free dim so the
                # tiny per-head matmuls stream back-to-back with batched copies
                kv_sb = hsb.tile([64, 8, 64], BF16, name="kv", tag="kv")
                for half in range(2):
                    kvp_ps = hps.tile([64, 256], F32, name="hps", tag="hps")
                    for i in range(4):
                        h = 4 * half + i
                        for kk in range(2):
                            nc.tensor.matmul(
                                kvp_ps[:, i * 64:(i + 1) * 64],
                                lhsT=atall[:, kk, h * 64:(h + 1) * 64],
                                rhs=wkv[:, kk, INNER + h * 64:INNER + (h + 1) * 64],
                                start=(kk == 0), stop=(kk == 1),
                            )
                    if half == 0:
                        nc.scalar.activation(out=kv_sb[:, 0:4, :], in_=kvp_ps[:],
                                             func=AF.Copy)
                    else:
                        nc.vector.tensor_copy(out=kv_sb[:, 4:8, :], in_=kvp_ps[:])
                for hp in range(4):
                    mtp_ps = hps.tile([128, C], F32, name="hps", tag="hps")
                    for i in range(2):
                        h = 2 * hp + i
                        nc.tensor.matmul(
                            mtp_ps[i * 64:(i + 1) * 64, :],
                            lhsT=kv_sb[:, h, :],
                            rhs=wq[:, h, :], start=True, stop=True)
                    if hp % 2 == 0:
                        nc.scalar.activation(out=MT_sb[:, hp, :], in_=mtp_ps[:],
                                             func=AF.Copy)
                    else:
                        nc.vector.tensor_copy(out=MT_sb[:, hp, :], in_=mtp_ps[:])

                # P^T = w_out @ M directly (lhsT = w_out^T = wo, rhs = M^T = MT_sb),
                # then PIT = S_ATTN * P^T + I -- no PE transpose round-trip needed.
                for cc in range(2):
                    p_ps = pps.tile([128, C], F32, name=f"p{cc}", tag=f"p{cc}")
                    for kk in range(4):
                        nc.tensor.matmul(
                            p_ps[:],
                            lhsT=wo[:, kk, cc * 128:(cc + 1) * 128],
                            rhs=MT_sb[:, kk, :],
                            start=(kk == 0), stop=(kk == 3),
                        )
                    for hb in range(2):
                        nc.vector.scalar_tensor_tensor(
                            out=PIT[:, cc, hb * 128:(hb + 1) * 128],
                            in0=p_ps[:, hb * 128:(hb + 1) * 128], scalar=S_ATTN,
                            in1=ident[:, cc, hb * 128:(hb + 1) * 128],
                            op0=ALU.mult, op1=ALU.add,
                        )
                # W1F = PI @ w1g^T and WLF = PI @ Wlin^T: fold the attention
                # apply into the MLP/output weights so x2 is never materialized.
                # oh-major order: tile 0's first MLP1 chunks need both cb
                # halves of oh=0, so emit those two folds first.
                for oh in range(2):
                    for cb in range(2):
                        wf_ps = hps.tile([128, 512], F32, name="wf", tag="hps")
                        for k2 in range(2):
                            nc.tensor.matmul(
                                wf_ps[:],
                                lhsT=PIT[:, k2, cb * 128:(cb + 1) * 128],
                                rhs=w1t[:, k2, oh * 512:(oh + 1) * 512],
                                start=(k2 == 0), stop=(k2 == 1),
                            )
                        if cb == 0:
                            nc.scalar.activation(
                                out=W1F[:, cb, oh * 512:(oh + 1) * 512],
                                in_=wf_ps[:], func=AF.Copy)
                        else:
                            nc.vector.tensor_copy(
                                out=W1F[:, cb, oh * 512:(oh + 1) * 512],
                                in_=wf_ps[:])
                for cb in range(2):
                    wl_ps = hps.tile([128, C], F32, name="wl", tag="hps")
                    for k2 in range(2):
                        nc.tensor.matmul(
                            wl_ps[:],
                            lhsT=PIT[:, k2, cb * 128:(cb + 1) * 128],
                            rhs=wlin[:, k2, :],
                            start=(k2 == 0), stop=(k2 == 1),
                        )
                    if cb == 0:
                        nc.scalar.activation(out=WLF[:, cb, :], in_=wl_ps[:],
                                             func=AF.Copy)
                    else:
                        nc.vector.tensor_copy(out=WLF[:, cb, :], in_=wl_ps[:])

            # ---------------- Phase 2: streamed MLP (attention pre-folded) ----------------
            with tc.tile_pool(name="gel", bufs=4) as gel, \
                 tc.tile_pool(name="nlp", bufs=2) as nlp, \
                 tc.tile_pool(name="yp", bufs=3) as yp, \
                 tc.tile_pool(name="mps", bufs=3, space="PSUM") as mps, \
                 tc.tile_pool(name="yps", bufs=2, space="PSUM") as yps:
                def emit_y_group(base, nl, cc, nh):
                    # y = x @ WLF + nl @ w2^T + b2f  (one [128,512] output group;
                    # groups of the previous tile are interleaved between this
                    # tile's MLP1 chunks so the PE and ACT paces stay matched)
                    sl = slice(base + nh * 512, base + (nh + 1) * 512)
                    y_ps = yps.tile([128, 512], F32, name="y", tag="y")
                    for k in range(2):
                        nc.tensor.matmul(
                            y_ps[:],
                            lhsT=WLF[:, k, cc * 128:(cc + 1) * 128],
                            rhs=xt_res[:, k, sl],
                            start=(k == 0), stop=False,
                        )
                    for kk in range(8):
                        nc.tensor.matmul(
                            y_ps[:],
                            lhsT=w2t[:, kk, cc * 128:(cc + 1) * 128],
                            rhs=nl[:, kk, nh, :],
                            start=False, stop=(kk == 7),
                        )
                    y_sb = yp.tile([128, 512], BF16, name="ysb", tag="ysb")
                    nc.scalar.activation(out=y_sb[:], in_=y_ps[:], func=AF.Identity,
                                         bias=bias[:, 10 + cc:11 + cc])
                    nc.sync.dma_start(out=yt_d[cc, :, sl], in_=y_sb[:])

                # tiles: (token base, 512-token subtile count); the first and
                # last macro tiles are split so the pipeline fills and drains
                # with less PE idle time
                TILES = ([(0, 1)] + [(512 + j * NT, 2) for j in range(NJ - 2)]
                         + [(6656, 1), (7168, 1), (7680, 1)])
                pending = None      # (base, nhc, nl, [groups left to emit])
                for ti, (base, nhc) in enumerate(TILES):
                    # MLP hidden + poly-gelu nonlinear part, one [128,nhc*512]
                    # chunk per MLP o-block (single per-partition bias each):
                    #   m  = x @ W1F + b1f      (PSUM, drained by ACT)
                    #   s  = 0.0012 m + 0.1972
                    #   t1 = m * s ; t2 = m^2 ; nl = t1 * t2
                    # j0 keeps Square off ACT (no y copies to hide it behind);
                    # j1 interleaves the previous tile's groups later because
                    # DVE is still catching up on tile 0's chains.
                    t2_act = () if ti == 0 else T2_ACT
                    t2_pool = (1, 2, 5, 6) if ti == 0 else T2_POOL
                    ilv = (3, 5, 7)
                    last = ti == len(TILES) - 1
                    nl = nlp.tile([128, 8, 2, 512], BF16, name="nl", tag="nl")
                    for o in range(8):
                        bcol = bias[:, 2 + o:3 + o]
                        m_ps = mps.tile([128, 2, 512], F32, name="m", tag="m")
                        for nh in range(nhc):
                            sl = slice(base + nh * 512, base + (nh + 1) * 512)
                            for k in range(2):
                                nc.tensor.matmul(
                                    m_ps[:, nh, :],
                                    lhsT=W1F[:, k, o * 128:(o + 1) * 128],
                                    rhs=xt_res[:, k, sl],
                                    start=(k == 0), stop=(k == 1),
                                )
                        ma = gel.tile([128, 2, 512], BF16, name="ma", tag="ma")
                        nc.scalar.activation(out=ma[:, 0:nhc, :], in_=m_ps[:, 0:nhc, :],
                                             func=AF.Identity, bias=bcol)
                        s = gel.tile([128, 2, 512], BF16, name="s", tag="s")
                        s_eng = nc.gpsimd if o in S_POOL else nc.vector
                        s_eng.tensor_scalar(out=s[:, 0:nhc, :], in0=ma[:, 0:nhc, :],
                                            scalar1=0.0012, scalar2=0.1972,
                                            op0=ALU.mult, op1=ALU.add)
                        t1 = gel.tile([128, 2, 512], BF16, name="t1", tag="t1")
                        nc.vector.tensor_tensor(out=t1[:, 0:nhc, :], in0=ma[:, 0:nhc, :],
                                                in1=s[:, 0:nhc, :], op=ALU.mult)
                        t2 = gel.tile([128, 2, 512], BF16, name="t2", tag="t2")
                        if o in t2_act:
                            nc.scalar.activation(out=t2[:, 0:nhc, :], in_=ma[:, 0:nhc, :],
                                                 func=AF.Square)
                        elif o in t2_pool:
                            nc.gpsimd.tensor_tensor(out=t2[:, 0:nhc, :], in0=ma[:, 0:nhc, :],
                                                    in1=ma[:, 0:nhc, :], op=ALU.mult)
                        else:
                            nc.vector.tensor_tensor(out=t2[:, 0:nhc, :], in0=ma[:, 0:nhc, :],
                                                    in1=ma[:, 0:nhc, :], op=ALU.mult)
                        nc.vector.tensor_tensor(out=nl[:, o, 0:nhc, :], in0=t1[:, 0:nhc, :],
                                                in1=t2[:, 0:nhc, :], op=ALU.mult)
                        if pending is not None and pending[3] and o in ilv:
                            g = pending[3].pop(0)
                            emit_y_group(pending[0], pending[2], *g)
                    if last:
                        # final 512 tokens: both groups emitted back-to-back,
                        # with the PSUM drains and stores on parallel engine
                        # paths (ACT+SP for cc=0, DVE+ACT-DGE for cc=1)
                        sl = slice(base, base + 512)
                        for cc in range(2):
                            y_ps = yps.tile([128, 512], F32, name="y", tag="y")
                            for k in range(2):
                                nc.tensor.matmul(
                                    y_ps[:],
                                    lhsT=WLF[:, k, cc * 128:(cc + 1) * 128],
                                    rhs=xt_res[:, k, sl],
                                    start=(k == 0), stop=False,
                                )
                            for kk in range(8):
                                nc.tensor.matmul(
                                    y_ps[:],
                                    lhsT=w2t[:, kk, cc * 128:(cc + 1) * 128],
                                    rhs=nl[:, kk, 0, :],
                                    start=False, stop=(kk == 7),
                                )
                            if cc == 0:
                                y_sb0 = yp.tile([128, 512], BF16, name="ysb", tag="ysb")
                                nc.scalar.activation(out=y_sb0[:], in_=y_ps[:],
                                                     func=AF.Identity,
                                                     bias=bias[:, 10:11])
                                nc.sync.dma_start(out=yt_d[0, :, sl], in_=y_sb0[:])
                            else:
                                y_sb1 = yp.tile([128, 512], BF16, name="ysb2", tag="ysb2")
                                nc.vector.tensor_scalar(out=y_sb1[:], in0=y_ps[:],
                                                        scalar1=bias[:, 11:12],
                                                        scalar2=None, op0=ALU.add)
                                nc.scalar.dma_start(out=yt_d[1, :, sl], in_=y_sb1[:])
                        break
                    if pending is not None:
                        for g in pending[3]:
                            emit_y_group(pending[0], pending[2], *g)
                    groups = [(cc, nh) for nh in range(nhc) for cc in (0, 1)]
                    pending = (base, nhc, nl, groups)

    _legalize_waits(nc, 1)
    return nc


def _get_program(reps=1):
    global _NC
    if reps != 1:
        return _build_program(reps)
    if _NC is None:
        _NC = _build_program()
    return _NC


def _prep_maps(x, gamma1, w_qkv, w_out, b_out, gamma2, w1, b1, w2, b2):
    f8 = np.float64
    x = np.asarray(x, np.float32)
    g1 = np.asarray(gamma1, f8)
    g2 = np.asarray(gamma2, f8)
    w_qkv = np.asarray(w_qkv, f8)
    w_out = np.asarray(w_out, f8)
    b_out = np.asarray(b_out, f8)
    w1 = np.asarray(w1, f8)
    b1 = np.asarray(b1, f8)
    w2 = np.asarray(w2, f8)
    b2 = np.asarray(b2, f8)

    wq = w_qkv[0:INNER] * g1[None, :]
    wk = w_qkv[INNER:2 * INNER] * g1[None, :]
    wv = w_qkv[2 * INNER:3 * INNER] * g1[None, :]
    w1g = w1 * g2[None, :]
    wlin_m = np.eye(C) + 0.5 * (w2 @ w1g)       # [c', c]
    b2v = b2 + 0.5 * (w2 @ b1)

    def pk(a, kdim):  # [kdim*128, F] -> [128, kdim, F]
        return np.ascontiguousarray(
            a.reshape(kdim, 128, a.shape[-1]).transpose(1, 0, 2)).astype(NP_BF16)

    wkvT = np.concatenate([wk.T, wv.T], axis=1)             # [256, 1024]
    wkv_h = pk(wkvT, 2)
    wq_h = np.ascontiguousarray(
        wq.reshape(H, 64, C).transpose(1, 0, 2)).astype(NP_BF16)
    wo_h = pk(w_out.T.copy(), 4)                            # [512,256]->[128,4,256]
    w1t_h = pk(w1g.T.copy(), 2)                             # [256,1024]->[128,2,1024]
    w2t_h = np.ascontiguousarray(
        w2.T.reshape(8, 128, C).transpose(1, 0, 2)).astype(NP_BF16)
    wlin_h = pk(wlin_m.T.copy(), 2)                         # [256,256]->[128,2,256]
    ident_h = pk(np.eye(C), 2)
    b1f = b_out @ w1g.T + b1                                # [1024]
    b2f = b2v + b_out @ wlin_m.T                            # [256]
    bias_h = np.concatenate([
        b_out.reshape(2, 128).T, b1f.reshape(8, 128).T, b2f.reshape(2, 128).T,
    ], axis=1).astype(np.float32)                           # [128, 12]
    shared = dict(wkv=wkv_h, wq=wq_h, wo=wo_h, w1t=w1t_h, w2t=w2t_h,
                  wlin=wlin_h, ident=ident_h, bias=bias_h)

    in_maps = []
    for b in range(B):
        xb = np.ascontiguousarray(x[b])
        # p-major fp8 layout for the DoubleRow Gram:
        # n = tb*2048 + u*256 + i*128 + p  ->  [p, tb, u*2+i, c]
        xq = np.ascontiguousarray(
            xb.reshape(4, 8, 2, 128, C).transpose(3, 0, 1, 2, 4)
            .reshape(128, 4, 16, C)).astype(NP_F8)
        in_maps.append(dict(xq=xq, xt=np.ascontiguousarray(xb.T).astype(NP_BF16),
                            **shared))
    return in_maps


def kernel(**inputs):
    global LAST_RESULTS
    nc = _get_program()
    in_maps = _prep_maps(**inputs)
    res = run_bass_kernel_spmd(nc, in_maps, list(range(N_CORES)))
    LAST_RESULTS = res
    out = np.empty((B, N, C), np.float32)
    for b in range(B):
        yt = np.asarray(res.results[b]["yt"]).astype(np.float32)   # [2, 128, N]
        out[b] = yt.reshape(C, N).T
    return out


# revision 67
# speedup vs baseline: 1.0142x; 1.0090x over previous
"""Trainium2 Bass kernel for nn_BobaTransformerBlock (dense transformer block
with linear attention + poly-gelu MLP), data-parallel over batch on 8 cores.

Math (per sample, exact reassociation of the reference):
  h  = x * g1
  Gx = x^T x                                   [256,256]   (fp8 DoubleRow)
  per head h: KV_h = wk'_h Gx wv'_h^T ; MT_h = KV_h^T wq'_h   (wX' = wX * g1)
  P  = (SCALE/N) * M @ w_out^T ;  PI = P + I
  x2 = x @ PI + b_out                          (attention + residual)
  m  = x @ W1F + b1f          (W1F = PI @ w1g^T, b1f = b_out @ w1g^T + b1)
  nl = (0.0012 m + 0.1972) * m * m^2           (poly-gelu nonlinear part)
  y  = x @ WLF + nl @ w2^T + b2f
       (WLF = PI @ Wlin^T, Wlin = I + 0.5 w2 @ w1g,
        b2f = b2 + 0.5 w2 @ b1 + b_out @ Wlin^T)

Device layout is channel-major ("transposed"): activations [c, n] so biases
are per-partition. Host supplies x twice: p-major fp8 (Gram phase, DoubleRow
pairs) and transposed bf16 (everything else). The device writes y transposed
in bf16; the host transposes/casts back during unsharding.

Phase 2 runs on 1024-token macro tiles (512 at the pipeline fill/drain
edges); per tile the 8 MLP o-chunks each produce m in a 2-bank PSUM tile
that ACT drains in one [128,1024] op (one per-partition bias per o-chunk).
The gelu chain is split across ACT (Square), DVE (4x tensor_scalar /
2x tensor_tensor), and GpSimd to balance engine busy time, and the
previous tile's y output groups are interleaved between this tile's MLP1
chunks so the PE and ACT paces stay matched (the steady state has zero
PE idle). All phase-1.5 PSUM drains are split across ACT and DVE to
halve the serial-latency chain; y stores issue from the SP HWDGE ring.
"""

import sys

for _p in ("/opt/trn_rl_repo", "/opt/pypackages"):
    if _p not in sys.path:
        sys.path.insert(0, _p)

from contextlib import ExitStack

import numpy as np

import concourse.bass as bass
import concourse.mybir as mybir
import concourse.tile as tile
from concourse.bass_utils import run_bass_kernel_spmd

F32 = mybir.dt.float32
BF16 = mybir.dt.bfloat16
F8 = mybir.dt.float8e4
NP_BF16 = mybir.dt.np(BF16)
NP_F8 = mybir.dt.np(F8)
AF = mybir.ActivationFunctionType
ALU = mybir.AluOpType
MPM = mybir.MatmulPerfMode

B, N, C = 8, 8192, 256
H, D = 8, 64
INNER = H * D          # 512
MLP = 4 * C            # 1024
SCALE = 1.0 / np.sqrt(D)
S_ATTN = float(SCALE / N)
N_CORES = 8
NT = 1024              # phase-2 macro tile (tokens)
NJ = N // NT           # 8

# elementwise engine assignment per o-chunk (0..7)
S_POOL = (0, 4)        # tensor_scalar s on GpSimd for these chunks
T2_ACT = (1, 5)        # m^2 via ACT Square-from-PSUM
T2_POOL = (2, 6)       # m^2 via GpSimd tensor_tensor

_NC = None             # cached Bass program
LAST_RESULTS = None    # BassKernelResults of the most recent run (for test.py)


def _legalize_waits(nc, max_waits=1):
    """walrus's TPB codegen accepts at most one sync wait per instruction.
    Move excess waits onto preceding same-engine NOPs."""
    ctr = 0
    for f in nc.m.functions:
        for bb in f.blocks:
            insts = bb.instructions
            i = 0
            while i < len(insts):
                inst = insts[i]
                si = inst.sync_info
                waits = list(si.on_wait) if (si is not None and si.on_wait) else []
                if len(waits) > max_waits:
                    keep = waits[-max_waits:]
                    extra = waits[:-max_waits]
                    pos = i
                    while extra:
                        chunk, extra = extra[:max_waits], extra[max_waits:]
                        nop = mybir.InstNoOp(
                            name=f"I-waitsplit-{ctr}",
                            engine=inst.engine,
                            ins=[],
                            outs=[],
                            sync_info=mybir.SyncInfo(on_wait=chunk, on_update=[]),
                        )
                        ctr += 1
                        insts.insert(pos, nop)
                        pos += 1
                        i += 1
                    inst.sync_info = mybir.SyncInfo(
                        on_wait=keep,
                        on_update=list(si.on_update) if si.on_update else [],
                    )
                i += 1
    return ctr


def _build_program(reps=1):
    nc = bass.Bass(trn_type="TRN2")

    xq_d = nc.declare_dram_parameter("xq", [128, 4, 16, 256], F8, isOutput=False)
    xt_d = nc.declare_dram_parameter("xt", [C, N], BF16, isOutput=False)
    wkv_d = nc.declare_dram_parameter("wkv", [128, 2, 2 * INNER], BF16, isOutput=False)
    wq_d = nc.declare_dram_parameter("wq", [64, H, C], BF16, isOutput=False)
    wo_d = nc.declare_dram_parameter("wo", [128, 4, C], BF16, isOutput=False)
    w1t_d = nc.declare_dram_parameter("w1t", [128, 2, MLP], BF16, isOutput=False)
    w2t_d = nc.declare_dram_parameter("w2t", [128, 8, C], BF16, isOutput=False)
    wlin_d = nc.declare_dram_parameter("wlin", [128, 2, C], BF16, isOutput=False)
    ident_d = nc.declare_dram_parameter("ident", [128, 2, C], BF16, isOutput=False)
    bias_d = nc.declare_dram_parameter("bias", [128, 12], F32, isOutput=False)
    yt_d = nc.declare_dram_parameter("yt", [2, 128, N], BF16, isOutput=True)

    with tile.TileContext(nc) as tc, ExitStack() as ctx:
        const = ctx.enter_context(tc.tile_pool(name="const", bufs=1))
        wkv = const.tile([128, 2, 2 * INNER], BF16, name="wkv", tag="wkv")
        wq = const.tile([64, H, C], BF16, name="wq", tag="wq")
        wo = const.tile([128, 4, C], BF16, name="wo", tag="wo")
        w1t = const.tile([128, 2, MLP], BF16, name="w1t", tag="w1t")
        w2t = const.tile([128, 8, C], BF16, name="w2t", tag="w2t")
        wlin = const.tile([128, 2, C], BF16, name="wlin", tag="wlin")
        ident = const.tile([128, 2, C], BF16, name="ident", tag="ident")
        bias = const.tile([128, 12], F32, name="bias", tag="bias")
        G_sb = const.tile([128, 2, C], BF16, name="G", tag="G")
        MT_sb = const.tile([128, 4, C], BF16, name="MT", tag="MT")
        xt_res = const.tile([128, 2, N], BF16, name="xt_res", tag="xt_res")
        PIT = const.tile([128, 2, C], BF16, name="PIT", tag="PIT")
        W1F = const.tile([128, 2, MLP], BF16, name="W1F", tag="W1F")
        WLF = const.tile([128, 2, C], BF16, name="WLF", tag="WLF")

        for _rep in range(reps):

            # ---------------- Phase 1: Gram matrix Gx = x^T x (fp8 DR) ----------------
            with tc.tile_pool(name="xp", bufs=4) as xp, \
                 tc.tile_pool(name="gps", bufs=1, space="PSUM") as gps:
                g_ps = [gps.tile([128, C], F32, name=f"g{k}", tag=f"g{k}") for k in range(2)]
                for tb in range(4):
                    if tb == 0:
                        # first chunk in two independent tiles so the Gram can
                        # start as soon as the first half-DMA lands
                        halves = []
                        for hh in range(2):
                            xh = xp.tile([128, 8, 256], F8, name="xh", tag=f"xh{hh}")
                            nc.sync.dma_start(out=xh[:],
                                              in_=xq_d[:, 0, hh * 8:(hh + 1) * 8, :])
                            halves.append(xh)
                        def xsl(u, ksl):
                            return halves[u // 4][:, 2 * (u % 4):2 * (u % 4) + 2, ksl]
                    else:
                        x_t = xp.tile([128, 16, 256], F8, name="x", tag="x")
                        nc.sync.dma_start(out=x_t[:], in_=xq_d[:, tb, :, :])
                        def xsl(u, ksl):
                            return x_t[:, 2 * u:2 * u + 2, ksl]
                    for u in range(8):
                        for k in range(2):
                            nc.tensor.matmul(
                                g_ps[k][:],
                                lhsT=xsl(u, slice(k * 128, (k + 1) * 128)),
                                rhs=xsl(u, slice(0, 256)),
                                start=(u == 0 and tb == 0),
                                stop=(u == 7 and tb == 3),
                                perf_mode=MPM.DoubleRow,
                            )
                # copies split across engines so each atall matmul's two G
                # chunks (k is its contraction side) land in parallel
                nc.scalar.activation(out=G_sb[:, 0, :], in_=g_ps[0][:], func=AF.Copy)
                nc.vector.tensor_copy(out=G_sb[:, 1, :], in_=g_ps[1][:])

            # Ordered input stream on the SP HWDGE ring (just-in-time): Gram xq
            # chunks were emitted above; phase-1.5 weights, then the phase-2
            # weights, then xt in per-macro-tile chunks.
            nc.sync.dma_start(out=wkv[:, :, 0:INNER], in_=wkv_d[:, :, 0:INNER])
            nc.sync.dma_start(out=wkv[:, :, INNER:2 * INNER],
                              in_=wkv_d[:, :, INNER:2 * INNER])
            for sb, dr in ((wq, wq_d), (wo, wo_d), (ident, ident_d),
                           (wlin, wlin_d), (w1t, w1t_d)):
                nc.sync.dma_start(out=sb[:], in_=dr[:])
            for k in range(2):
                nc.sync.dma_start(out=xt_res[:, k, 0:NT],
                                  in_=xt_d[k * 128:(k + 1) * 128, 0:NT])
            for sb, dr in ((bias, bias_d), (w2t, w2t_d)):
                nc.sync.dma_start(out=sb[:], in_=dr[:])
            for J in range(1, NJ):
                for k in range(2):
                    nc.sync.dma_start(
                        out=xt_res[:, k, J * NT:(J + 1) * NT],
                        in_=xt_d[k * 128:(k + 1) * 128, J * NT:(J + 1) * NT])

            # ---------------- Phase 1.5: per-head KV path -> PI -> folds ----------------
            with tc.tile_pool(name="hsb", bufs=6) as hsb, \
                 tc.tile_pool(name="hps", bufs=6, space="PSUM") as hps, \
                 tc.tile_pool(name="pps", bufs=1, space="PSUM") as pps:
                # ATall = Gx @ wk'^T for all heads at once (Gx is symmetric, so
                # no transpose of the intermediate is ever needed)
                atall = hsb.tile([128, 2, INNER], BF16, name="atall", tag="atall")
                for cc in range(2):
                    at_ps = hps.tile([128, INNER], F32, name="hps", tag="hps")
                    for k2 in range(2):
                        nc.tensor.matmul(
                            at_ps[:],
                            lhsT=G_sb[:, k2, cc * 128:(cc + 1) * 128],
                            rhs=wkv[:, k2, 0:INNER],
                            start=(k2 == 0), stop=(k2 == 1),
                        )
                    if cc == 0:
                        nc.scalar.activation(out=atall[:, cc, :], in_=at_ps[:],
                                             func=AF.Copy)
                    else:
                        nc.vector.tensor_copy(out=atall[:, cc, :], in_=at_ps[:])

                # all 8 heads batched: KV_h side by side in the free dim so the
                # tiny per-head matmuls stream back-to-back with batched copies
                kv_sb = hsb.tile([64, 8, 64], BF16, name="kv", tag="kv")
                for half in range(2):
                    kvp_ps = hps.tile([64, 256], F32, name="hps", tag="hps")
                    for i in range(4):
                        h = 4 * half + i
                        for kk in range(2):
                            nc.tensor.matmul(
                                kvp_ps[:, i * 64:(i + 1) * 64],
                                lhsT=atall[:, kk, h * 64:(h + 1) * 64],
                                rhs=wkv[:, kk, INNER + h * 64:INNER + (h + 1) * 64],
                                start=(kk == 0), stop=(kk == 1),
                            )
                    if half == 0:
                        nc.scalar.activation(out=kv_sb[:, 0:4, :], in_=kvp_ps[:],
                                             func=AF.Copy)
                    else:
                        nc.vector.tensor_copy(out=kv_sb[:, 4:8, :], in_=kvp_ps[:])
                for hp in range(4):
                    mtp_ps = hps.tile([128, C], F32, name="hps", tag="hps")
                    for i in range(2):
                        h = 2 * hp + i
                        nc.tensor.matmul(
                            mtp_ps[i * 64:(i + 1) * 64, :],
                            lhsT=kv_sb[:, h, :],
                            rhs=wq[:, h, :], start=True, stop=True)
                    if hp % 2 == 0:
                        nc.scalar.activation(out=MT_sb[:, hp, :], in_=mtp_ps[:],
                                             func=AF.Copy)
                    else:
                        nc.vector.tensor_copy(out=MT_sb[:, hp, :], in_=mtp_ps[:])

                # P^T = w_out @ M directly (lhsT = w_out^T = wo, rhs = M^T = MT_sb),
                # then PIT = S_ATTN * P^T + I -- no PE transpose round-trip needed.
                for cc in range(2):
                    p_ps = pps.tile([128, C], F32, name=f"p{cc}", tag=f"p{cc}")
                    for kk in range(4):
                        nc.tensor.matmul(
                            p_ps[:],
                            lhsT=wo[:, kk, cc * 128:(cc + 1) * 128],
                            rhs=MT_sb[:, kk, :],
                            start=(kk == 0), stop=(kk == 3),
                        )
                    for hb in range(2):
                        nc.vector.scalar_tensor_tensor(
                            out=PIT[:, cc, hb * 128:(hb + 1) * 128],
                            in0=p_ps[:, hb * 128:(hb + 1) * 128], scalar=S_ATTN,
                            in1=ident[:, cc, hb * 128:(hb + 1) * 128],
                            op0=ALU.mult, op1=ALU.add,
                        )
                # W1F = PI @ w1g^T and WLF = PI @ Wlin^T: fold the attention
                # apply into the MLP/output weights so x2 is never materialized.
                # oh-major order: tile 0's first MLP1 chunks need both cb
                # halves of oh=0, so emit those two folds first.
                for oh in range(2):
                    for cb in range(2):
                        wf_ps = hps.tile([128, 512], F32, name="wf", tag="hps")
                        for k2 in range(2):
                            nc.tensor.matmul(
                                wf_ps[:],
                                lhsT=PIT[:, k2, cb * 128:(cb + 1) * 128],
                                rhs=w1t[:, k2, oh * 512:(oh + 1) * 512],
                                start=(k2 == 0), stop=(k2 == 1),
                            )
                        if cb == 0:
                            nc.scalar.activation(
                                out=W1F[:, cb, oh * 512:(oh + 1) * 512],
                                in_=wf_ps[:], func=AF.Copy)
                        else:
                            nc.vector.tensor_copy(
                                out=W1F[:, cb, oh * 512:(oh + 1) * 512],
                                in_=wf_ps[:])
                for cb in range(2):
                    wl_ps = hps.tile([128, C], F32, name="wl", tag="hps")
                    for k2 in range(2):
                        nc.tensor.matmul(
                            wl_ps[:],
                            lhsT=PIT[:, k2, cb * 128:(cb + 1) * 128],
                            rhs=wlin[:, k2, :],
                            start=(k2 == 0), stop=(k2 == 1),
                        )
                    if cb == 0:
                        nc.scalar.activation(out=WLF[:, cb, :], in_=wl_ps[:],
                                             func=AF.Copy)
                    else:
                        nc.vector.tensor_copy(out=WLF[:, cb, :], in_=wl_ps[:])

            # ---------------- Phase 2: streamed MLP (attention pre-folded) ----------------
            with tc.tile_pool(name="gel", bufs=8) as gel, \
                 tc.tile_pool(name="nlp", bufs=4) as nlp, \
                 tc.tile_pool(name="yp", bufs=4) as yp, \
                 tc.tile_pool(name="mps", bufs=3, space="PSUM") as mps, \
                 tc.tile_pool(name="yps", bufs=2, space="PSUM") as yps:
                def emit_y_group(base, nl, cc, nh):
                    # y = x @ WLF + nl @ w2^T + b2f  (one [128,512] output group;
                    # groups of the previous tile are interleaved between this
                    # tile's MLP1 chunks so the PE and ACT paces stay matched)
                    sl = slice(base + nh * 512, base + (nh + 1) * 512)
                    y_ps = yps.tile([128, 512], F32, name="y", tag="y")
                    for k in range(2):
                        nc.tensor.matmul(
                            y_ps[:],
                            lhsT=WLF[:, k, cc * 128:(cc + 1) * 128],
                            rhs=xt_res[:, k, sl],
                            start=(k == 0), stop=False,
                        )
                    for kk in range(8):
                        nc.tensor.matmul(
                            y_ps[:],
                            lhsT=w2t[:, kk, cc * 128:(cc + 1) * 128],
                            rhs=nl[:, kk, nh, :],
                            start=False, stop=(kk == 7),
                        )
                    y_sb = yp.tile([128, 512], BF16, name="ysb", tag="ysb")
                    nc.scalar.activation(out=y_sb[:], in_=y_ps[:], func=AF.Identity,
                                         bias=bias[:, 10 + cc:11 + cc])
                    nc.sync.dma_start(out=yt_d[cc, :, sl], in_=y_sb[:])

                # tiles: (token base, 512-token subtile count); the first and
                # last macro tiles are split so the pipeline fills and drains
                # with less PE idle time
                TILES = ([(0, 1)] + [(512 + j * NT, 2) for j in range(NJ - 2)]
                         + [(6656, 1), (7168, 1), (7680, 1)])
                pending = None      # (base, nhc, nl, [groups left to emit])
                for ti, (base, nhc) in enumerate(TILES):
                    # MLP hidden + poly-gelu nonlinear part, one [128,nhc*512]
                    # chunk per MLP o-block (single per-partition bias each):
                    #   m  = x @ W1F + b1f      (PSUM, drained by ACT)
                    #   s  = 0.0012 m + 0.1972
                    #   t1 = m * s ; t2 = m^2 ; nl = t1 * t2
                    # j0 keeps Square off ACT (no y copies to hide it behind);
                    # j1 interleaves the previous tile's groups later because
                    # DVE is still catching up on tile 0's chains.
                    t2_act = () if ti == 0 else T2_ACT
                    t2_pool = (1, 2, 5, 6) if ti == 0 else T2_POOL
                    ilv = (3, 5, 7)
                    last = ti == len(TILES) - 1
                    nl = nlp.tile([128, 8, 2, 512], BF16, name="nl", tag="nl")
                    for o in range(8):
                        bcol = bias[:, 2 + o:3 + o]
                        m_ps = mps.tile([128, 2, 512], F32, name="m", tag="m")
                        for nh in range(nhc):
                            sl = slice(base + nh * 512, base + (nh + 1) * 512)
                            for k in range(2):
                                nc.tensor.matmul(
                                    m_ps[:, nh, :],
                                    lhsT=W1F[:, k, o * 128:(o + 1) * 128],
                                    rhs=xt_res[:, k, sl],
                                    start=(k == 0), stop=(k == 1),
                                )
                        ma = gel.tile([128, 2, 512], BF16, name="ma", tag="ma")
                        nc.scalar.activation(out=ma[:, 0:nhc, :], in_=m_ps[:, 0:nhc, :],
                                             func=AF.Identity, bias=bcol)
                        s = gel.tile([128, 2, 512], BF16, name="s", tag="s")
                        s_eng = nc.gpsimd if o in S_POOL else nc.vector
                        s_eng.tensor_scalar(out=s[:, 0:nhc, :], in0=ma[:, 0:nhc, :],
                                            scalar1=0.0012, scalar2=0.1972,
                                            op0=ALU.mult, op1=ALU.add)
                        t1 = gel.tile([128, 2, 512], BF16, name="t1", tag="t1")
                        nc.vector.tensor_tensor(out=t1[:, 0:nhc, :], in0=ma[:, 0:nhc, :],
                                                in1=s[:, 0:nhc, :], op=ALU.mult)
                        t2 = gel.tile([128, 2, 512], BF16, name="t2", tag="t2")
                        if o in t2_act:
                            nc.scalar.activation(out=t2[:, 0:nhc, :], in_=ma[:, 0:nhc, :],
                                                 func=AF.Square)
                        elif o in t2_pool:
                            nc.gpsimd.tensor_tensor(out=t2[:, 0:nhc, :], in0=ma[:, 0:nhc, :],
                                                    in1=ma[:, 0:nhc, :], op=ALU.mult)
                        else:
                            nc.vector.tensor_tensor(out=t2[:, 0:nhc, :], in0=ma[:, 0:nhc, :],
                                                    in1=ma[:, 0:nhc, :], op=ALU.mult)
                        nc.vector.tensor_tensor(out=nl[:, o, 0:nhc, :], in0=t1[:, 0:nhc, :],
                                                in1=t2[:, 0:nhc, :], op=ALU.mult)
                        if pending is not None and pending[3] and o in ilv:
                            g = pending[3].pop(0)
                            emit_y_group(pending[0], pending[2], *g)
                    if last:
                        # final 512 tokens: both groups emitted back-to-back,
                        # with the PSUM drains and stores on parallel engine
                        # paths (ACT+SP for cc=0, DVE+ACT-DGE for cc=1)
                        sl = slice(base, base + 512)
                        for cc in range(2):
                            y_ps = yps.tile([128, 512], F32, name="y", tag="y")
                            for k in range(2):
                                nc.tensor.matmul(
                                    y_ps[:],
                                    lhsT=WLF[:, k, cc * 128:(cc + 1) * 128],
                                    rhs=xt_res[:, k, sl],
                                    start=(k == 0), stop=False,
                                )
                            for kk in range(8):
                                nc.tensor.matmul(
                                    y_ps[:],
                                    lhsT=w2t[:, kk, cc * 128:(cc + 1) * 128],
                                    rhs=nl[:, kk, 0, :],
                                    start=False, stop=(kk == 7),
                                )
                            if cc == 0:
                                y_sb0 = yp.tile([128, 512], BF16, name="ysb", tag="ysb")
                                nc.scalar.activation(out=y_sb0[:], in_=y_ps[:],
                                                     func=AF.Identity,
                                                     bias=bias[:, 10:11])
                                nc.sync.dma_start(out=yt_d[0, :, sl], in_=y_sb0[:])
                            else:
                                y_sb1 = yp.tile([128, 512], BF16, name="ysb2", tag="ysb2")
                                nc.vector.tensor_scalar(out=y_sb1[:], in0=y_ps[:],
                                                        scalar1=bias[:, 11:12],
                                                        scalar2=None, op0=ALU.add)
                                nc.scalar.dma_start(out=yt_d[1, :, sl], in_=y_sb1[:])
                        break
                    if pending is not None:
                        for g in pending[3]:
                            emit_y_group(pending[0], pending[2], *g)
                    groups = [(cc, nh) for nh in range(nhc) for cc in (0, 1)]
                    pending = (base, nhc, nl, groups)

    _legalize_waits(nc, 1)
    return nc


def _get_program(reps=1):
    global _NC
    if reps != 1:
        return _build_program(reps)
    if _NC is None:
        _NC = _build_program()
    return _NC


def _prep_maps(x, gamma1, w_qkv, w_out, b_out, gamma2, w1, b1, w2, b2):
    f8 = np.float64
    x = np.asarray(x, np.float32)
    g1 = np.asarray(gamma1, f8)
    g2 = np.asarray(gamma2, f8)
    w_qkv = np.asarray(w_qkv, f8)
    w_out = np.asarray(w_out, f8)
    b_out = np.asarray(b_out, f8)
    w1 = np.asarray(w1, f8)
    b1 = np.asarray(b1, f8)
    w2 = np.asarray(w2, f8)
    b2 = np.asarray(b2, f8)

    wq = w_qkv[0:INNER] * g1[None, :]
    wk = w_qkv[INNER:2 * INNER] * g1[None, :]
    wv = w_qkv[2 * INNER:3 * INNER] * g1[None, :]
    w1g = w1 * g2[None, :]
    wlin_m = np.eye(C) + 0.5 * (w2 @ w1g)       # [c', c]
    b2v = b2 + 0.5 * (w2 @ b1)

    def pk(a, kdim):  # [kdim*128, F] -> [128, kdim, F]
        return np.ascontiguousarray(
            a.reshape(kdim, 128, a.shape[-1]).transpose(1, 0, 2)).astype(NP_BF16)

    wkvT = np.concatenate([wk.T, wv.T], axis=1)             # [256, 1024]
    wkv_h = pk(wkvT, 2)
    wq_h = np.ascontiguousarray(
        wq.reshape(H, 64, C).transpose(1, 0, 2)).astype(NP_BF16)
    wo_h = pk(w_out.T.copy(), 4)                            # [512,256]->[128,4,256]
    w1t_h = pk(w1g.T.copy(), 2)                             # [256,1024]->[128,2,1024]
    w2t_h = np.ascontiguousarray(
        w2.T.reshape(8, 128, C).transpose(1, 0, 2)).astype(NP_BF16)
    wlin_h = pk(wlin_m.T.copy(), 2)                         # [256,256]->[128,2,256]
    ident_h = pk(np.eye(C), 2)
    b1f = b_out @ w1g.T + b1                                # [1024]
    b2f = b2v + b_out @ wlin_m.T                            # [256]
    bias_h = np.concatenate([
        b_out.reshape(2, 128).T, b1f.reshape(8, 128).T, b2f.reshape(2, 128).T,
    ], axis=1).astype(np.float32)                           # [128, 12]
    shared = dict(wkv=wkv_h, wq=wq_h, wo=wo_h, w1t=w1t_h, w2t=w2t_h,
                  wlin=wlin_h, ident=ident_h, bias=bias_h)

    in_maps = []
    for b in range(B):
        xb = np.ascontiguousarray(x[b])
        # p-major fp8 layout for the DoubleRow Gram:
        # n = tb*2048 + u*256 + i*128 + p  ->  [p, tb, u*2+i, c]
        xq = np.ascontiguousarray(
            xb.reshape(4, 8, 2, 128, C).transpose(3, 0, 1, 2, 4)
            .reshape(128, 4, 16, C)).astype(NP_F8)
        in_maps.append(dict(xq=xq, xt=np.ascontiguousarray(xb.T).astype(NP_BF16),
                            **shared))
    return in_maps


def kernel(**inputs):
    global LAST_RESULTS
    nc = _get_program()
    in_maps = _prep_maps(**inputs)
    res = run_bass_kernel_spmd(nc, in_maps, list(range(N_CORES)))
    LAST_RESULTS = res
    out = np.empty((B, N, C), np.float32)
    for b in range(B):
        yt = np.asarray(res.results[b]["yt"]).astype(np.float32)   # [2, 128, N]
        out[b] = yt.reshape(C, N).T
    return out


# revision 75
# speedup vs baseline: 1.0156x; 1.0014x over previous
"""Trainium2 Bass kernel for nn_BobaTransformerBlock (dense transformer block
with linear attention + poly-gelu MLP), data-parallel over batch on 8 cores.

Math (per sample, exact reassociation of the reference):
  h  = x * g1
  Gx = x^T x                                   [256,256]   (fp8 DoubleRow)
  per head h: KV_h = wk'_h Gx wv'_h^T ; MT_h = KV_h^T wq'_h   (wX' = wX * g1)
  P  = (SCALE/N) * M @ w_out^T ;  PI = P + I
  x2 = x @ PI + b_out                          (attention + residual)
  m  = x @ W1F + b1f          (W1F = PI @ w1g^T, b1f = b_out @ w1g^T + b1)
  nl = (0.0012 m + 0.1972) * m * m^2           (poly-gelu nonlinear part)
  y  = x @ WLF + nl @ w2^T + b2f
       (WLF = PI @ Wlin^T, Wlin = I + 0.5 w2 @ w1g,
        b2f = b2 + 0.5 w2 @ b1 + b_out @ Wlin^T)

Device layout is channel-major ("transposed"): activations [c, n] so biases
are per-partition. Host supplies x twice: p-major fp8 (Gram phase, DoubleRow
pairs) and transposed bf16 (everything else). The device writes y transposed
in bf16; the host transposes/casts back during unsharding.

Phase 2 runs on 1024-token macro tiles (512 at the pipeline fill/drain
edges); per tile the 8 MLP o-chunks each produce m in a 2-bank PSUM tile
that ACT drains in one [128,1024] op (one per-partition bias per o-chunk).
The gelu chain is split across ACT (Square), DVE (4x tensor_scalar /
2x tensor_tensor), and GpSimd to balance engine busy time, and the
previous tile's y output groups are interleaved between this tile's MLP1
chunks so the PE and ACT paces stay matched (the steady state has zero
PE idle). All phase-1.5 PSUM drains are split across ACT and DVE to
halve the serial-latency chain; y stores issue from the SP HWDGE ring.
"""

import sys

for _p in ("/opt/trn_rl_repo", "/opt/pypackages"):
    if _p not in sys.path:
        sys.path.insert(0, _p)

from contextlib import ExitStack

import numpy as np

import concourse.bass as bass
import concourse.mybir as mybir
import concourse.tile as tile
from concourse.bass_utils import run_bass_kernel_spmd

F32 = mybir.dt.float32
BF16 = mybir.dt.bfloat16
F8 = mybir.dt.float8e4
NP_BF16 = mybir.dt.np(BF16)
NP_F8 = mybir.dt.np(F8)
AF = mybir.ActivationFunctionType
ALU = mybir.AluOpType
MPM = mybir.MatmulPerfMode

B, N, C = 8, 8192, 256
H, D = 8, 64
INNER = H * D          # 512
MLP = 4 * C            # 1024
SCALE = 1.0 / np.sqrt(D)
S_ATTN = float(SCALE / N)
N_CORES = 8
NT = 1024              # phase-2 macro tile (tokens)
NJ = N // NT           # 8

# elementwise engine assignment per o-chunk (0..7)
S_POOL = (0, 4)        # tensor_scalar s on GpSimd for these chunks
T2_ACT = (1, 5)        # m^2 via ACT Square-from-PSUM
T2_POOL = (2, 6)       # m^2 via GpSimd tensor_tensor

_NC = None             # cached Bass program
LAST_RESULTS = None    # BassKernelResults of the most recent run (for test.py)


def _legalize_waits(nc, max_waits=1):
    """walrus's TPB codegen accepts at most one sync wait per instruction.
    Move excess waits onto preceding same-engine NOPs."""
    ctr = 0
    for f in nc.m.functions:
        for bb in f.blocks:
            insts = bb.instructions
            i = 0
            while i < len(insts):
                inst = insts[i]
                si = inst.sync_info
                waits = list(si.on_wait) if (si is not None and si.on_wait) else []
                if len(waits) > max_waits:
                    keep = waits[-max_waits:]
                    extra = waits[:-max_waits]
                    pos = i
                    while extra:
                        chunk, extra = extra[:max_waits], extra[max_waits:]
                        nop = mybir.InstNoOp(
                            name=f"I-waitsplit-{ctr}",
                            engine=inst.engine,
                            ins=[],
                            outs=[],
                            sync_info=mybir.SyncInfo(on_wait=chunk, on_update=[]),
                        )
                        ctr += 1
                        insts.insert(pos, nop)
                        pos += 1
                        i += 1
                    inst.sync_info = mybir.SyncInfo(
                        on_wait=keep,
                        on_update=list(si.on_update) if si.on_update else [],
                    )
                i += 1
    return ctr


def _build_program(reps=1):
    nc = bass.Bass(trn_type="TRN2")

    xq_d = nc.declare_dram_parameter("xq", [128, 4, 16, 256], F8, isOutput=False)
    xt_d = nc.declare_dram_parameter("xt", [C, N], BF16, isOutput=False)
    wkv_d = nc.declare_dram_parameter("wkv", [128, 2, 2 * INNER], BF16, isOutput=False)
    wq_d = nc.declare_dram_parameter("wq", [64, H, C], BF16, isOutput=False)
    wo_d = nc.declare_dram_parameter("wo", [128, 4, C], BF16, isOutput=False)
    w1t_d = nc.declare_dram_parameter("w1t", [128, 2, MLP], BF16, isOutput=False)
    w2t_d = nc.declare_dram_parameter("w2t", [128, 8, C], BF16, isOutput=False)
    wlin_d = nc.declare_dram_parameter("wlin", [128, 2, C], BF16, isOutput=False)
    ident_d = nc.declare_dram_parameter("ident", [128, 2, C], BF16, isOutput=False)
    bias_d = nc.declare_dram_parameter("bias", [128, 12], F32, isOutput=False)
    yt_d = nc.declare_dram_parameter("yt", [2, 128, N], BF16, isOutput=True)

    with tile.TileContext(nc) as tc, ExitStack() as ctx:
        const = ctx.enter_context(tc.tile_pool(name="const", bufs=1))
        wkv = const.tile([128, 2, 2 * INNER], BF16, name="wkv", tag="wkv")
        wq = const.tile([64, H, C], BF16, name="wq", tag="wq")
        wo = const.tile([128, 4, C], BF16, name="wo", tag="wo")
        w1t = const.tile([128, 2, MLP], BF16, name="w1t", tag="w1t")
        w2t = const.tile([128, 8, C], BF16, name="w2t", tag="w2t")
        wlin = const.tile([128, 2, C], BF16, name="wlin", tag="wlin")
        ident = const.tile([128, 2, C], BF16, name="ident", tag="ident")
        bias = const.tile([128, 12], F32, name="bias", tag="bias")
        G_sb = const.tile([128, 2, C], BF16, name="G", tag="G")
        MT_sb = const.tile([128, 4, C], BF16, name="MT", tag="MT")
        xt_res = const.tile([128, 2, N], BF16, name="xt_res", tag="xt_res")
        PIT = const.tile([128, 2, C], BF16, name="PIT", tag="PIT")
        W1F = const.tile([128, 2, MLP], BF16, name="W1F", tag="W1F")
        WLF = const.tile([128, 2, C], BF16, name="WLF", tag="WLF")

        for _rep in range(reps):

            # ---------------- Phase 1: Gram matrix Gx = x^T x (fp8 DR) ----------------
            with tc.tile_pool(name="xp", bufs=4) as xp, \
                 tc.tile_pool(name="gps", bufs=1, space="PSUM") as gps:
                g_ps = [gps.tile([128, C], F32, name=f"g{k}", tag=f"g{k}") for k in range(2)]
                for tb in range(4):
                    if tb == 0:
                        # first chunk in two independent tiles so the Gram can
                        # start as soon as the first half-DMA lands
                        halves = []
                        for hh in range(2):
                            xh = xp.tile([128, 8, 256], F8, name="xh", tag=f"xh{hh}")
                            nc.sync.dma_start(out=xh[:],
                                              in_=xq_d[:, 0, hh * 8:(hh + 1) * 8, :])
                            halves.append(xh)
                        def xsl(u, ksl):
                            return halves[u // 4][:, 2 * (u % 4):2 * (u % 4) + 2, ksl]
                    else:
                        x_t = xp.tile([128, 16, 256], F8, name="x", tag="x")
                        nc.sync.dma_start(out=x_t[:], in_=xq_d[:, tb, :, :])
                        def xsl(u, ksl):
                            return x_t[:, 2 * u:2 * u + 2, ksl]
                    for u in range(8):
                        for k in range(2):
                            nc.tensor.matmul(
                                g_ps[k][:],
                                lhsT=xsl(u, slice(k * 128, (k + 1) * 128)),
                                rhs=xsl(u, slice(0, 256)),
                                start=(u == 0 and tb == 0),
                                stop=(u == 7 and tb == 3),
                                perf_mode=MPM.DoubleRow,
                            )
                # copies split across engines so each atall matmul's two G
                # chunks (k is its contraction side) land in parallel
                nc.scalar.activation(out=G_sb[:, 0, :], in_=g_ps[0][:], func=AF.Copy)
                nc.vector.tensor_copy(out=G_sb[:, 1, :], in_=g_ps[1][:])

            # Ordered input stream on the SP HWDGE ring (just-in-time): Gram xq
            # chunks were emitted above; phase-1.5 weights, then the phase-2
            # weights, then xt in per-macro-tile chunks.
            nc.sync.dma_start(out=wkv[:, :, 0:INNER], in_=wkv_d[:, :, 0:INNER])
            nc.sync.dma_start(out=wkv[:, :, INNER:2 * INNER],
                              in_=wkv_d[:, :, INNER:2 * INNER])
            for sb, dr in ((wq, wq_d), (wo, wo_d), (ident, ident_d),
                           (wlin, wlin_d), (w1t, w1t_d)):
                nc.sync.dma_start(out=sb[:], in_=dr[:])
            for k in range(2):
                nc.sync.dma_start(out=xt_res[:, k, 0:NT],
                                  in_=xt_d[k * 128:(k + 1) * 128, 0:NT])
            for sb, dr in ((bias, bias_d), (w2t, w2t_d)):
                nc.sync.dma_start(out=sb[:], in_=dr[:])
            for J in range(1, NJ):
                for k in range(2):
                    nc.sync.dma_start(
                        out=xt_res[:, k, J * NT:(J + 1) * NT],
                        in_=xt_d[k * 128:(k + 1) * 128, J * NT:(J + 1) * NT])

            # ---------------- Phase 1.5: per-head KV path -> PI -> folds ----------------
            with tc.tile_pool(name="hsb", bufs=6) as hsb, \
                 tc.tile_pool(name="hps", bufs=6, space="PSUM") as hps, \
                 tc.tile_pool(name="pps", bufs=1, space="PSUM") as pps:
                # ATall = Gx @ wk'^T for all heads at once (Gx is symmetric, so
                # no transpose of the intermediate is ever needed)
                atall = hsb.tile([128, 2, INNER], BF16, name="atall", tag="atall")
                for cc in range(2):
                    at_ps = hps.tile([128, INNER], F32, name="hps", tag="hps")
                    for k2 in range(2):
                        nc.tensor.matmul(
                            at_ps[:],
                            lhsT=G_sb[:, k2, cc * 128:(cc + 1) * 128],
                            rhs=wkv[:, k2, 0:INNER],
                            start=(k2 == 0), stop=(k2 == 1),
                        )
                    for pc in range(2):
                        psl = slice(pc * 256, (pc + 1) * 256)
                        if cc == 0:
                            nc.scalar.activation(out=atall[:, cc, psl],
                                                 in_=at_ps[:, psl], func=AF.Copy)
                        else:
                            nc.vector.tensor_copy(out=atall[:, cc, psl],
                                                  in_=at_ps[:, psl])

                # all 8 heads batched: KV_h side by side in the free dim so the
                # tiny per-head matmuls stream back-to-back with batched copies
                kv_sb = hsb.tile([64, 8, 64], BF16, name="kv", tag="kv")
                for half in range(2):
                    kvp_ps = hps.tile([64, 256], F32, name="hps", tag="hps")
                    for i in range(4):
                        h = 4 * half + i
                        for kk in range(2):
                            nc.tensor.matmul(
                                kvp_ps[:, i * 64:(i + 1) * 64],
                                lhsT=atall[:, kk, h * 64:(h + 1) * 64],
                                rhs=wkv[:, kk, INNER + h * 64:INNER + (h + 1) * 64],
                                start=(kk == 0), stop=(kk == 1),
                            )
                    if half == 0:
                        nc.scalar.activation(out=kv_sb[:, 0:4, :], in_=kvp_ps[:],
                                             func=AF.Copy)
                    else:
                        nc.vector.tensor_copy(out=kv_sb[:, 4:8, :], in_=kvp_ps[:])
                for hp in range(4):
                    mtp_ps = hps.tile([128, C], F32, name="hps", tag="hps")
                    for i in range(2):
                        h = 2 * hp + i
                        nc.tensor.matmul(
                            mtp_ps[i * 64:(i + 1) * 64, :],
                            lhsT=kv_sb[:, h, :],
                            rhs=wq[:, h, :], start=True, stop=True)
                    if hp % 2 == 0:
                        nc.scalar.activation(out=MT_sb[:, hp, :], in_=mtp_ps[:],
                                             func=AF.Copy)
                    else:
                        nc.vector.tensor_copy(out=MT_sb[:, hp, :], in_=mtp_ps[:])

                # P^T = w_out @ M directly (lhsT = w_out^T = wo, rhs = M^T = MT_sb),
                # then PIT = S_ATTN * P^T + I -- no PE transpose round-trip needed.
                for cc in range(2):
                    p_ps = pps.tile([128, C], F32, name=f"p{cc}", tag=f"p{cc}")
                    for kk in range(4):
                        nc.tensor.matmul(
                            p_ps[:],
                            lhsT=wo[:, kk, cc * 128:(cc + 1) * 128],
                            rhs=MT_sb[:, kk, :],
                            start=(kk == 0), stop=(kk == 3),
                        )
                    for hb in range(2):
                        nc.vector.scalar_tensor_tensor(
                            out=PIT[:, cc, hb * 128:(hb + 1) * 128],
                            in0=p_ps[:, hb * 128:(hb + 1) * 128], scalar=S_ATTN,
                            in1=ident[:, cc, hb * 128:(hb + 1) * 128],
                            op0=ALU.mult, op1=ALU.add,
                        )
                # W1F = PI @ w1g^T and WLF = PI @ Wlin^T: fold the attention
                # apply into the MLP/output weights so x2 is never materialized.
                # oh-major order: tile 0's first MLP1 chunks need both cb
                # halves of oh=0, so emit those two folds first.
                for oh in range(2):
                    for cb in range(2):
                        wf_ps = hps.tile([128, 512], F32, name="wf", tag="hps")
                        for k2 in range(2):
                            nc.tensor.matmul(
                                wf_ps[:],
                                lhsT=PIT[:, k2, cb * 128:(cb + 1) * 128],
                                rhs=w1t[:, k2, oh * 512:(oh + 1) * 512],
                                start=(k2 == 0), stop=(k2 == 1),
                            )
                        if cb == 0:
                            nc.scalar.activation(
                                out=W1F[:, cb, oh * 512:(oh + 1) * 512],
                                in_=wf_ps[:], func=AF.Copy)
                        else:
                            nc.vector.tensor_copy(
                                out=W1F[:, cb, oh * 512:(oh + 1) * 512],
                                in_=wf_ps[:])
                for cb in range(2):
                    wl_ps = hps.tile([128, C], F32, name="wl", tag="hps")
                    for k2 in range(2):
                        nc.tensor.matmul(
                            wl_ps[:],
                            lhsT=PIT[:, k2, cb * 128:(cb + 1) * 128],
                            rhs=wlin[:, k2, :],
                            start=(k2 == 0), stop=(k2 == 1),
                        )
                    if cb == 0:
                        nc.scalar.activation(out=WLF[:, cb, :], in_=wl_ps[:],
                                             func=AF.Copy)
                    else:
                        nc.vector.tensor_copy(out=WLF[:, cb, :], in_=wl_ps[:])

            # ---------------- Phase 2: streamed MLP (attention pre-folded) ----------------
            with tc.tile_pool(name="gel", bufs=8) as gel, \
                 tc.tile_pool(name="nlp", bufs=4) as nlp, \
                 tc.tile_pool(name="yp", bufs=4) as yp, \
                 tc.tile_pool(name="mps", bufs=3, space="PSUM") as mps, \
                 tc.tile_pool(name="yps", bufs=2, space="PSUM") as yps:
                def emit_y_group(base, nl, cc, nh):
                    # y = x @ WLF + nl @ w2^T + b2f  (one [128,512] output group;
                    # groups of the previous tile are interleaved between this
                    # tile's MLP1 chunks so the PE and ACT paces stay matched)
                    sl = slice(base + nh * 512, base + (nh + 1) * 512)
                    y_ps = yps.tile([128, 512], F32, name="y", tag="y")
                    for k in range(2):
                        nc.tensor.matmul(
                            y_ps[:],
                            lhsT=WLF[:, k, cc * 128:(cc + 1) * 128],
                            rhs=xt_res[:, k, sl],
                            start=(k == 0), stop=False,
                        )
                    for kk in range(8):
                        nc.tensor.matmul(
                            y_ps[:],
                            lhsT=w2t[:, kk, cc * 128:(cc + 1) * 128],
                            rhs=nl[:, kk, nh, :],
                            start=False, stop=(kk == 7),
                        )
                    y_sb = yp.tile([128, 512], BF16, name="ysb", tag="ysb")
                    nc.scalar.activation(out=y_sb[:], in_=y_ps[:], func=AF.Identity,
                                         bias=bias[:, 10 + cc:11 + cc])
                    nc.sync.dma_start(out=yt_d[cc, :, sl], in_=y_sb[:])

                # tiles: (token base, 512-token subtile count); the first and
                # last macro tiles are split so the pipeline fills and drains
                # with less PE idle time
                TILES = ([(0, 1)] + [(512 + j * NT, 2) for j in range(NJ - 2)]
                         + [(6656, 1), (7168, 1), (7680, 1)])
                pending = None      # (base, nhc, nl, [groups left to emit])
                for ti, (base, nhc) in enumerate(TILES):
                    # MLP hidden + poly-gelu nonlinear part, one [128,nhc*512]
                    # chunk per MLP o-block (single per-partition bias each):
                    #   m  = x @ W1F + b1f      (PSUM, drained by ACT)
                    #   s  = 0.0012 m + 0.1972
                    #   t1 = m * s ; t2 = m^2 ; nl = t1 * t2
                    # j0 keeps Square off ACT (no y copies to hide it behind);
                    # j1 interleaves the previous tile's groups later because
                    # DVE is still catching up on tile 0's chains.
                    t2_act = () if ti == 0 else T2_ACT
                    t2_pool = (1, 2, 5, 6) if ti == 0 else T2_POOL
                    ilv = (3, 5, 7)
                    last = ti == len(TILES) - 1
                    nl = nlp.tile([128, 8, 2, 512], BF16, name="nl", tag="nl")
                    for o in range(8):
                        bcol = bias[:, 2 + o:3 + o]
                        m_ps = mps.tile([128, 2, 512], F32, name="m", tag="m")
                        for nh in range(nhc):
                            sl = slice(base + nh * 512, base + (nh + 1) * 512)
                            for k in range(2):
                                nc.tensor.matmul(
                                    m_ps[:, nh, :],
                                    lhsT=W1F[:, k, o * 128:(o + 1) * 128],
                                    rhs=xt_res[:, k, sl],
                                    start=(k == 0), stop=(k == 1),
                                )
                        ma = gel.tile([128, 2, 512], BF16, name="ma", tag="ma")
                        nc.scalar.activation(out=ma[:, 0:nhc, :], in_=m_ps[:, 0:nhc, :],
                                             func=AF.Identity, bias=bcol)
                        s = gel.tile([128, 2, 512], BF16, name="s", tag="s")
                        s_eng = nc.gpsimd if o in S_POOL else nc.vector
                        s_eng.tensor_scalar(out=s[:, 0:nhc, :], in0=ma[:, 0:nhc, :],
                                            scalar1=0.0012, scalar2=0.1972,
                                            op0=ALU.mult, op1=ALU.add)
                        t1 = gel.tile([128, 2, 512], BF16, name="t1", tag="t1")
                        nc.vector.tensor_tensor(out=t1[:, 0:nhc, :], in0=ma[:, 0:nhc, :],
                                                in1=s[:, 0:nhc, :], op=ALU.mult)
                        t2 = gel.tile([128, 2, 512], BF16, name="t2", tag="t2")
                        if o in t2_act:
                            nc.scalar.activation(out=t2[:, 0:nhc, :], in_=ma[:, 0:nhc, :],
                                                 func=AF.Square)
                        elif o in t2_pool:
                            nc.gpsimd.tensor_tensor(out=t2[:, 0:nhc, :], in0=ma[:, 0:nhc, :],
                                                    in1=ma[:, 0:nhc, :], op=ALU.mult)
                        else:
                            nc.vector.tensor_tensor(out=t2[:, 0:nhc, :], in0=ma[:, 0:nhc, :],
                                                    in1=ma[:, 0:nhc, :], op=ALU.mult)
                        nc.vector.tensor_tensor(out=nl[:, o, 0:nhc, :], in0=t1[:, 0:nhc, :],
                                                in1=t2[:, 0:nhc, :], op=ALU.mult)
                        if pending is not None and pending[3] and o in ilv:
                            g = pending[3].pop(0)
                            emit_y_group(pending[0], pending[2], *g)
                    if last:
                        # final 512 tokens: both groups emitted back-to-back,
                        # with the PSUM drains and stores on parallel engine
                        # paths (ACT+SP for cc=0, DVE+ACT-DGE for cc=1)
                        sl = slice(base, base + 512)
                        for cc in range(2):
                            y_ps = yps.tile([128, 512], F32, name="y", tag="y")
                            for k in range(2):
                                nc.tensor.matmul(
                                    y_ps[:],
                                    lhsT=WLF[:, k, cc * 128:(cc + 1) * 128],
                                    rhs=xt_res[:, k, sl],
                                    start=(k == 0), stop=False,
                                )
                            for kk in range(8):
                                nc.tensor.matmul(
                                    y_ps[:],
                                    lhsT=w2t[:, kk, cc * 128:(cc + 1) * 128],
                                    rhs=nl[:, kk, 0, :],
                                    start=False, stop=(kk == 7),
                                )
                            if cc == 0:
                                y_sb0 = yp.tile([128, 512], BF16, name="ysb", tag="ysb")
                                nc.scalar.activation(out=y_sb0[:], in_=y_ps[:],
                                                     func=AF.Identity,
                                                     bias=bias[:, 10:11])
                                nc.sync.dma_start(out=yt_d[0, :, sl], in_=y_sb0[:])
                            else:
                                y_sb1 = yp.tile([128, 512], BF16, name="ysb2", tag="ysb2")
                                nc.vector.tensor_scalar(out=y_sb1[:], in0=y_ps[:],
                                                        scalar1=bias[:, 11:12],
                                                        scalar2=None, op0=ALU.add)
                                nc.scalar.dma_start(out=yt_d[1, :, sl], in_=y_sb1[:])
                        break
                    if pending is not None:
                        for g in pending[3]:
                            emit_y_group(pending[0], pending[2], *g)
                    groups = [(cc, nh) for nh in range(nhc) for cc in (0, 1)]
                    pending = (base, nhc, nl, groups)

    _legalize_waits(nc, 1)
    return nc


def _get_program(reps=1):
    global _NC
    if reps != 1:
        return _build_program(reps)
    if _NC is None:
        _NC = _build_program()
    return _NC


def _prep_maps(x, gamma1, w_qkv, w_out, b_out, gamma2, w1, b1, w2, b2):
    f8 = np.float64
    x = np.asarray(x, np.float32)
    g1 = np.asarray(gamma1, f8)
    g2 = np.asarray(gamma2, f8)
    w_qkv = np.asarray(w_qkv, f8)
    w_out = np.asarray(w_out, f8)
    b_out = np.asarray(b_out, f8)
    w1 = np.asarray(w1, f8)
    b1 = np.asarray(b1, f8)
    w2 = np.asarray(w2, f8)
    b2 = np.asarray(b2, f8)

    wq = w_qkv[0:INNER] * g1[None, :]
    wk = w_qkv[INNER:2 * INNER] * g1[None, :]
    wv = w_qkv[2 * INNER:3 * INNER] * g1[None, :]
    w1g = w1 * g2[None, :]
    wlin_m = np.eye(C) + 0.5 * (w2 @ w1g)       # [c', c]
    b2v = b2 + 0.5 * (w2 @ b1)

    def pk(a, kdim):  # [kdim*128, F] -> [128, kdim, F]
        return np.ascontiguousarray(
            a.reshape(kdim, 128, a.shape[-1]).transpose(1, 0, 2)).astype(NP_BF16)

    wkvT = np.concatenate([wk.T, wv.T], axis=1)             # [256, 1024]
    wkv_h = pk(wkvT, 2)
    wq_h = np.ascontiguousarray(
        wq.reshape(H, 64, C).transpose(1, 0, 2)).astype(NP_BF16)
    wo_h = pk(w_out.T.copy(), 4)                            # [512,256]->[128,4,256]
    w1t_h = pk(w1g.T.copy(), 2)                             # [256,1024]->[128,2,1024]
    w2t_h = np.ascontiguousarray(
        w2.T.reshape(8, 128, C).transpose(1, 0, 2)).astype(NP_BF16)
    wlin_h = pk(wlin_m.T.copy(), 2)                         # [256,256]->[128,2,256]
    ident_h = pk(np.eye(C), 2)
    b1f = b_out @ w1g.T + b1                                # [1024]
    b2f = b2v + b_out @ wlin_m.T                            # [256]
    bias_h = np.concatenate([
        b_out.reshape(2, 128).T, b1f.reshape(8, 128).T, b2f.reshape(2, 128).T,
    ], axis=1).astype(np.float32)                           # [128, 12]
    shared = dict(wkv=wkv_h, wq=wq_h, wo=wo_h, w1t=w1t_h, w2t=w2t_h,
                  wlin=wlin_h, ident=ident_h, bias=bias_h)

    in_maps = []
    for b in range(B):
        xb = np.ascontiguousarray(x[b])
        # p-major fp8 layout for the DoubleRow Gram:
        # n = tb*2048 + u*256 + i*128 + p  ->  [p, tb, u*2+i, c]
        xq = np.ascontiguousarray(
            xb.reshape(4, 8, 2, 128, C).transpose(3, 0, 1, 2, 4)
            .reshape(128, 4, 16, C)).astype(NP_F8)
        in_maps.append(dict(xq=xq, xt=np.ascontiguousarray(xb.T).astype(NP_BF16),
                            **shared))
    return in_maps


def kernel(**inputs):
    global LAST_RESULTS
    nc = _get_program()
    in_maps = _prep_maps(**inputs)
    res = run_bass_kernel_spmd(nc, in_maps, list(range(N_CORES)))
    LAST_RESULTS = res
    out = np.empty((B, N, C), np.float32)
    for b in range(B):
        yt = np.asarray(res.results[b]["yt"]).astype(np.float32)   # [2, 128, N]
        out[b] = yt.reshape(C, N).T
    return out


# revision 79
# speedup vs baseline: 1.0175x; 1.0018x over previous
"""Trainium2 Bass kernel for nn_BobaTransformerBlock (dense transformer block
with linear attention + poly-gelu MLP), data-parallel over batch on 8 cores.

Math (per sample, exact reassociation of the reference):
  h  = x * g1
  Gx = x^T x                                   [256,256]   (fp8 DoubleRow)
  per head h: KV_h = wk'_h Gx wv'_h^T ; MT_h = KV_h^T wq'_h   (wX' = wX * g1)
  P  = (SCALE/N) * M @ w_out^T ;  PI = P + I
  x2 = x @ PI + b_out                          (attention + residual)
  m  = x @ W1F + b1f          (W1F = PI @ w1g^T, b1f = b_out @ w1g^T + b1)
  nl = (0.0012 m + 0.1972) * m * m^2           (poly-gelu nonlinear part)
  y  = x @ WLF + nl @ w2^T + b2f
       (WLF = PI @ Wlin^T, Wlin = I + 0.5 w2 @ w1g,
        b2f = b2 + 0.5 w2 @ b1 + b_out @ Wlin^T)

Device layout is channel-major ("transposed"): activations [c, n] so biases
are per-partition. Host supplies x twice: p-major fp8 (Gram phase, DoubleRow
pairs) and transposed bf16 (everything else). The device writes y transposed
in bf16; the host transposes/casts back during unsharding.

Phase 2 runs on 1024-token macro tiles (512 at the pipeline fill/drain
edges); per tile the 8 MLP o-chunks each produce m in a 2-bank PSUM tile
that ACT drains in one [128,1024] op (one per-partition bias per o-chunk).
The gelu chain is split across ACT (Square), DVE (4x tensor_scalar /
2x tensor_tensor), and GpSimd to balance engine busy time, and the
previous tile's y output groups are interleaved between this tile's MLP1
chunks so the PE and ACT paces stay matched (the steady state has zero
PE idle). All phase-1.5 PSUM drains are split across ACT and DVE to
halve the serial-latency chain; y stores issue from the SP HWDGE ring.
"""

import sys

for _p in ("/opt/trn_rl_repo", "/opt/pypackages"):
    if _p not in sys.path:
        sys.path.insert(0, _p)

from contextlib import ExitStack

import numpy as np

import concourse.bass as bass
import concourse.mybir as mybir
import concourse.tile as tile
from concourse.bass_utils import run_bass_kernel_spmd

F32 = mybir.dt.float32
BF16 = mybir.dt.bfloat16
F8 = mybir.dt.float8e4
NP_BF16 = mybir.dt.np(BF16)
NP_F8 = mybir.dt.np(F8)
AF = mybir.ActivationFunctionType
ALU = mybir.AluOpType
MPM = mybir.MatmulPerfMode

B, N, C = 8, 8192, 256
H, D = 8, 64
INNER = H * D          # 512
MLP = 4 * C            # 1024
SCALE = 1.0 / np.sqrt(D)
S_ATTN = float(SCALE / N)
N_CORES = 8
NT = 1024              # phase-2 macro tile (tokens)
NJ = N // NT           # 8

# elementwise engine assignment per o-chunk (0..7)
S_POOL = (0, 4)        # tensor_scalar s on GpSimd for these chunks
T2_ACT = (1, 5)        # m^2 via ACT Square-from-PSUM
T2_POOL = (2, 6)       # m^2 via GpSimd tensor_tensor

_NC = None             # cached Bass program
LAST_RESULTS = None    # BassKernelResults of the most recent run (for test.py)


def _legalize_waits(nc, max_waits=1):
    """walrus's TPB codegen accepts at most one sync wait per instruction.
    Move excess waits onto preceding same-engine NOPs."""
    ctr = 0
    for f in nc.m.functions:
        for bb in f.blocks:
            insts = bb.instructions
            i = 0
            while i < len(insts):
                inst = insts[i]
                si = inst.sync_info
                waits = list(si.on_wait) if (si is not None and si.on_wait) else []
                if len(waits) > max_waits:
                    keep = waits[-max_waits:]
                    extra = waits[:-max_waits]
                    pos = i
                    while extra:
                        chunk, extra = extra[:max_waits], extra[max_waits:]
                        nop = mybir.InstNoOp(
                            name=f"I-waitsplit-{ctr}",
                            engine=inst.engine,
                            ins=[],
                            outs=[],
                            sync_info=mybir.SyncInfo(on_wait=chunk, on_update=[]),
                        )
                        ctr += 1
                        insts.insert(pos, nop)
                        pos += 1
                        i += 1
                    inst.sync_info = mybir.SyncInfo(
                        on_wait=keep,
                        on_update=list(si.on_update) if si.on_update else [],
                    )
                i += 1
    return ctr


def _build_program(reps=1):
    nc = bass.Bass(trn_type="TRN2")

    xq_d = nc.declare_dram_parameter("xq", [128, 4, 16, 256], F8, isOutput=False)
    xt_d = nc.declare_dram_parameter("xt", [C, N], BF16, isOutput=False)
    wkv_d = nc.declare_dram_parameter("wkv", [128, 2, 2 * INNER], BF16, isOutput=False)
    wq_d = nc.declare_dram_parameter("wq", [64, H, C], BF16, isOutput=False)
    wo_d = nc.declare_dram_parameter("wo", [128, 4, C], BF16, isOutput=False)
    w1t_d = nc.declare_dram_parameter("w1t", [128, 2, MLP], BF16, isOutput=False)
    w2t_d = nc.declare_dram_parameter("w2t", [128, 8, C], BF16, isOutput=False)
    wlin_d = nc.declare_dram_parameter("wlin", [128, 2, C], BF16, isOutput=False)
    ident_d = nc.declare_dram_parameter("ident", [128, 2, C], BF16, isOutput=False)
    bias_d = nc.declare_dram_parameter("bias", [128, 12], F32, isOutput=False)
    yt_d = nc.declare_dram_parameter("yt", [2, 128, N], BF16, isOutput=True)

    with tile.TileContext(nc) as tc, ExitStack() as ctx:
        const = ctx.enter_context(tc.tile_pool(name="const", bufs=1))
        wkv = const.tile([128, 2, 2 * INNER], BF16, name="wkv", tag="wkv")
        wq = const.tile([64, H, C], BF16, name="wq", tag="wq")
        wo = const.tile([128, 4, C], BF16, name="wo", tag="wo")
        w1t = const.tile([128, 2, MLP], BF16, name="w1t", tag="w1t")
        w2t = const.tile([128, 8, C], BF16, name="w2t", tag="w2t")
        wlin = const.tile([128, 2, C], BF16, name="wlin", tag="wlin")
        ident = const.tile([128, 2, C], BF16, name="ident", tag="ident")
        bias = const.tile([128, 12], F32, name="bias", tag="bias")
        G_sb = const.tile([128, 2, C], BF16, name="G", tag="G")
        MT_sb = const.tile([128, 4, C], BF16, name="MT", tag="MT")
        xt_res = const.tile([128, 2, N], BF16, name="xt_res", tag="xt_res")
        PIT = const.tile([128, 2, C], BF16, name="PIT", tag="PIT")
        W1F = const.tile([128, 2, MLP], BF16, name="W1F", tag="W1F")
        WLF = const.tile([128, 2, C], BF16, name="WLF", tag="WLF")

        for _rep in range(reps):

            # ---------------- Phase 1: Gram matrix Gx = x^T x (fp8 DR) ----------------
            with tc.tile_pool(name="xp", bufs=4) as xp, \
                 tc.tile_pool(name="gps", bufs=1, space="PSUM") as gps:
                g_ps = [gps.tile([128, C], F32, name=f"g{k}", tag=f"g{k}") for k in range(2)]
                for tb in range(4):
                    if tb == 0:
                        # first chunk in two independent tiles so the Gram can
                        # start as soon as the first half-DMA lands
                        halves = []
                        for hh in range(2):
                            xh = xp.tile([128, 8, 256], F8, name="xh", tag=f"xh{hh}")
                            nc.sync.dma_start(out=xh[:],
                                              in_=xq_d[:, 0, hh * 8:(hh + 1) * 8, :])
                            halves.append(xh)
                        def xsl(u, ksl):
                            return halves[u // 4][:, 2 * (u % 4):2 * (u % 4) + 2, ksl]
                    else:
                        x_t = xp.tile([128, 16, 256], F8, name="x", tag="x")
                        nc.sync.dma_start(out=x_t[:], in_=xq_d[:, tb, :, :])
                        def xsl(u, ksl):
                            return x_t[:, 2 * u:2 * u + 2, ksl]
                    for u in range(8):
                        for k in range(2):
                            nc.tensor.matmul(
                                g_ps[k][:],
                                lhsT=xsl(u, slice(k * 128, (k + 1) * 128)),
                                rhs=xsl(u, slice(0, 256)),
                                start=(u == 0 and tb == 0),
                                stop=(u == 7 and tb == 3),
                                perf_mode=MPM.DoubleRow,
                            )
                # copies split across engines so each atall matmul's two G
                # chunks (k is its contraction side) land in parallel
                nc.scalar.activation(out=G_sb[:, 0, :], in_=g_ps[0][:], func=AF.Copy)
                nc.vector.tensor_copy(out=G_sb[:, 1, :], in_=g_ps[1][:])

            # Ordered input stream on the SP HWDGE ring (just-in-time): Gram xq
            # chunks were emitted above; phase-1.5 weights, then the phase-2
            # weights, then xt in per-macro-tile chunks.
            nc.sync.dma_start(out=wkv[:, :, 0:INNER], in_=wkv_d[:, :, 0:INNER])
            nc.sync.dma_start(out=wkv[:, :, INNER:2 * INNER],
                              in_=wkv_d[:, :, INNER:2 * INNER])
            for sb, dr in ((wq, wq_d), (wo, wo_d), (ident, ident_d),
                           (wlin, wlin_d), (w1t, w1t_d)):
                nc.sync.dma_start(out=sb[:], in_=dr[:])
            for k in range(2):
                nc.sync.dma_start(out=xt_res[:, k, 0:NT],
                                  in_=xt_d[k * 128:(k + 1) * 128, 0:NT])
            for sb, dr in ((bias, bias_d), (w2t, w2t_d)):
                nc.sync.dma_start(out=sb[:], in_=dr[:])
            for J in range(1, NJ):
                for k in range(2):
                    nc.sync.dma_start(
                        out=xt_res[:, k, J * NT:(J + 1) * NT],
                        in_=xt_d[k * 128:(k + 1) * 128, J * NT:(J + 1) * NT])

            # ---------------- Phase 1.5: per-head KV path -> PI -> folds ----------------
            with tc.tile_pool(name="hsb", bufs=6) as hsb, \
                 tc.tile_pool(name="hps", bufs=6, space="PSUM") as hps, \
                 tc.tile_pool(name="pps", bufs=1, space="PSUM") as pps:
                # ATall = Gx @ wk'^T for all heads at once (Gx is symmetric, so
                # no transpose of the intermediate is ever needed)
                atall = hsb.tile([128, 2, INNER], BF16, name="atall", tag="atall")
                for cc in range(2):
                    at_ps = hps.tile([128, INNER], F32, name="hps", tag="hps")
                    for k2 in range(2):
                        nc.tensor.matmul(
                            at_ps[:],
                            lhsT=G_sb[:, k2, cc * 128:(cc + 1) * 128],
                            rhs=wkv[:, k2, 0:INNER],
                            start=(k2 == 0), stop=(k2 == 1),
                        )
                    for pc in range(2):
                        psl = slice(pc * 256, (pc + 1) * 256)
                        if cc == 0:
                            nc.scalar.activation(out=atall[:, cc, psl],
                                                 in_=at_ps[:, psl], func=AF.Copy)
                        else:
                            nc.vector.tensor_copy(out=atall[:, cc, psl],
                                                  in_=at_ps[:, psl])

                # all 8 heads batched: KV_h side by side in the free dim so the
                # tiny per-head matmuls stream back-to-back with batched copies
                kv_sb = hsb.tile([64, 8, 64], BF16, name="kv", tag="kv")
                for half in range(2):
                    kvp_ps = hps.tile([64, 256], F32, name="hps", tag="hps")
                    for i in range(4):
                        h = 4 * half + i
                        for kk in range(2):
                            nc.tensor.matmul(
                                kvp_ps[:, i * 64:(i + 1) * 64],
                                lhsT=atall[:, kk, h * 64:(h + 1) * 64],
                                rhs=wkv[:, kk, INNER + h * 64:INNER + (h + 1) * 64],
                                start=(kk == 0), stop=(kk == 1),
                            )
                    if half == 0:
                        nc.scalar.activation(out=kv_sb[:, 0:4, :], in_=kvp_ps[:],
                                             func=AF.Copy)
                    else:
                        nc.vector.tensor_copy(out=kv_sb[:, 4:8, :], in_=kvp_ps[:])
                for hp in range(4):
                    mtp_ps = hps.tile([128, C], F32, name="hps", tag="hps")
                    for i in range(2):
                        h = 2 * hp + i
                        nc.tensor.matmul(
                            mtp_ps[i * 64:(i + 1) * 64, :],
                            lhsT=kv_sb[:, h, :],
                            rhs=wq[:, h, :], start=True, stop=True)
                    if hp % 2 == 0:
                        nc.scalar.activation(out=MT_sb[:, hp, :], in_=mtp_ps[:],
                                             func=AF.Copy)
                    else:
                        nc.vector.tensor_copy(out=MT_sb[:, hp, :], in_=mtp_ps[:])

                # P^T = w_out @ M directly (lhsT = w_out^T = wo, rhs = M^T = MT_sb),
                # then PIT = S_ATTN * P^T + I -- no PE transpose round-trip needed.
                for cc in range(2):
                    p_ps = pps.tile([128, C], F32, name=f"p{cc}", tag=f"p{cc}")
                    for kk in range(4):
                        nc.tensor.matmul(
                            p_ps[:],
                            lhsT=wo[:, kk, cc * 128:(cc + 1) * 128],
                            rhs=MT_sb[:, kk, :],
                            start=(kk == 0), stop=(kk == 3),
                        )
                    for hb in range(2):
                        nc.vector.scalar_tensor_tensor(
                            out=PIT[:, cc, hb * 128:(hb + 1) * 128],
                            in0=p_ps[:, hb * 128:(hb + 1) * 128], scalar=S_ATTN,
                            in1=ident[:, cc, hb * 128:(hb + 1) * 128],
                            op0=ALU.mult, op1=ALU.add,
                        )
                # W1F = PI @ w1g^T and WLF = PI @ Wlin^T: fold the attention
                # apply into the MLP/output weights so x2 is never materialized.
                # oh-major order: tile 0's first MLP1 chunks need both cb
                # halves of oh=0, so emit those two folds first.
                for oh in range(2):
                    for cb in range(2):
                        wf_ps = hps.tile([128, 512], F32, name="wf", tag="hps")
                        for k2 in range(2):
                            nc.tensor.matmul(
                                wf_ps[:],
                                lhsT=PIT[:, k2, cb * 128:(cb + 1) * 128],
                                rhs=w1t[:, k2, oh * 512:(oh + 1) * 512],
                                start=(k2 == 0), stop=(k2 == 1),
                            )
                        if cb == 0:
                            nc.scalar.activation(
                                out=W1F[:, cb, oh * 512:(oh + 1) * 512],
                                in_=wf_ps[:], func=AF.Copy)
                        else:
                            nc.vector.tensor_copy(
                                out=W1F[:, cb, oh * 512:(oh + 1) * 512],
                                in_=wf_ps[:])
                for cb in range(2):
                    wl_ps = hps.tile([128, C], F32, name="wl", tag="hps")
                    for k2 in range(2):
                        nc.tensor.matmul(
                            wl_ps[:],
                            lhsT=PIT[:, k2, cb * 128:(cb + 1) * 128],
                            rhs=wlin[:, k2, :],
                            start=(k2 == 0), stop=(k2 == 1),
                        )
                    if cb == 0:
                        nc.scalar.activation(out=WLF[:, cb, :], in_=wl_ps[:],
                                             func=AF.Copy)
                    else:
                        nc.vector.tensor_copy(out=WLF[:, cb, :], in_=wl_ps[:])

            # ---------------- Phase 2: streamed MLP (attention pre-folded) ----------------
            with tc.tile_pool(name="gel", bufs=8) as gel, \
                 tc.tile_pool(name="nlp", bufs=4) as nlp, \
                 tc.tile_pool(name="yp", bufs=4) as yp, \
                 tc.tile_pool(name="mps", bufs=3, space="PSUM") as mps, \
                 tc.tile_pool(name="yps", bufs=2, space="PSUM") as yps:
                def emit_y_group(base, nl, cc, nh):
                    # y = x @ WLF + nl @ w2^T + b2f  (one [128,512] output group;
                    # groups of the previous tile are interleaved between this
                    # tile's MLP1 chunks so the PE and ACT paces stay matched)
                    sl = slice(base + nh * 512, base + (nh + 1) * 512)
                    y_ps = yps.tile([128, 512], F32, name="y", tag="y")
                    for k in range(2):
                        nc.tensor.matmul(
                            y_ps[:],
                            lhsT=WLF[:, k, cc * 128:(cc + 1) * 128],
                            rhs=xt_res[:, k, sl],
                            start=(k == 0), stop=False,
                        )
                    for kk in range(8):
                        nc.tensor.matmul(
                            y_ps[:],
                            lhsT=w2t[:, kk, cc * 128:(cc + 1) * 128],
                            rhs=nl[:, kk, nh, :],
                            start=False, stop=(kk == 7),
                        )
                    y_sb = yp.tile([128, 512], BF16, name="ysb", tag="ysb")
                    nc.scalar.activation(out=y_sb[:], in_=y_ps[:], func=AF.Identity,
                                         bias=bias[:, 10 + cc:11 + cc])
                    nc.sync.dma_start(out=yt_d[cc, :, sl], in_=y_sb[:])

                # tiles: (token base, 512-token subtile count); the first and
                # last macro tiles are split so the pipeline fills and drains
                # with less PE idle time
                TILES = ([(0, 1)] + [(512 + j * NT, 2) for j in range(NJ - 2)]
                         + [(6656, 1), (7168, 1), (7680, 1)])
                pending = None      # (base, nhc, nl, [groups left to emit])
                for ti, (base, nhc) in enumerate(TILES):
                    # MLP hidden + poly-gelu nonlinear part, one [128,nhc*512]
                    # chunk per MLP o-block (single per-partition bias each):
                    #   m  = x @ W1F + b1f      (PSUM, drained by ACT)
                    #   s  = 0.0012 m + 0.1972
                    #   t1 = m * s ; t2 = m^2 ; nl = t1 * t2
                    # j0 keeps Square off ACT (no y copies to hide it behind);
                    # j1 interleaves the previous tile's groups later because
                    # DVE is still catching up on tile 0's chains.
                    t2_act = () if ti == 0 else T2_ACT
                    t2_pool = (1, 2, 5, 6) if ti == 0 else T2_POOL
                    ilv = (3, 5, 7)
                    last = ti == len(TILES) - 1
                    nl = nlp.tile([128, 8, 2, 512], BF16, name="nl", tag="nl")
                    for o in range(8):
                        bcol = bias[:, 2 + o:3 + o]
                        if ti == 0 and o % 2 == 1:
                            # tile 0 has no y work yet: borrow the idle y PSUM
                            # banks to deepen the MLP1 lookahead during fill
                            m_ps = yps.tile([128, 1, 512], F32, name="y", tag="y")
                        else:
                            m_ps = mps.tile([128, 2, 512], F32, name="m", tag="m")
                        for nh in range(nhc):
                            sl = slice(base + nh * 512, base + (nh + 1) * 512)
                            for k in range(2):
                                nc.tensor.matmul(
                                    m_ps[:, nh, :],
                                    lhsT=W1F[:, k, o * 128:(o + 1) * 128],
                                    rhs=xt_res[:, k, sl],
                                    start=(k == 0), stop=(k == 1),
                                )
                        ma = gel.tile([128, 2, 512], BF16, name="ma", tag="ma")
                        nc.scalar.activation(out=ma[:, 0:nhc, :], in_=m_ps[:, 0:nhc, :],
                                             func=AF.Identity, bias=bcol)
                        s = gel.tile([128, 2, 512], BF16, name="s", tag="s")
                        s_eng = nc.gpsimd if o in S_POOL else nc.vector
                        s_eng.tensor_scalar(out=s[:, 0:nhc, :], in0=ma[:, 0:nhc, :],
                                            scalar1=0.0012, scalar2=0.1972,
                                            op0=ALU.mult, op1=ALU.add)
                        t1 = gel.tile([128, 2, 512], BF16, name="t1", tag="t1")
                        nc.vector.tensor_tensor(out=t1[:, 0:nhc, :], in0=ma[:, 0:nhc, :],
                                                in1=s[:, 0:nhc, :], op=ALU.mult)
                        t2 = gel.tile([128, 2, 512], BF16, name="t2", tag="t2")
                        if o in t2_act:
                            nc.scalar.activation(out=t2[:, 0:nhc, :], in_=ma[:, 0:nhc, :],
                                                 func=AF.Square)
                        elif o in t2_pool:
                            nc.gpsimd.tensor_tensor(out=t2[:, 0:nhc, :], in0=ma[:, 0:nhc, :],
                                                    in1=ma[:, 0:nhc, :], op=ALU.mult)
                        else:
                            nc.vector.tensor_tensor(out=t2[:, 0:nhc, :], in0=ma[:, 0:nhc, :],
                                                    in1=ma[:, 0:nhc, :], op=ALU.mult)
                        nc.vector.tensor_tensor(out=nl[:, o, 0:nhc, :], in0=t1[:, 0:nhc, :],
                                                in1=t2[:, 0:nhc, :], op=ALU.mult)
                        if pending is not None and pending[3] and o in ilv:
                            g = pending[3].pop(0)
                            emit_y_group(pending[0], pending[2], *g)
                    if last:
                        # final 512 tokens: both groups emitted back-to-back,
                        # with the PSUM drains and stores on parallel engine
                        # paths (ACT+SP for cc=0, DVE+ACT-DGE for cc=1)
                        sl = slice(base, base + 512)
                        for cc in range(2):
                            y_ps = yps.tile([128, 512], F32, name="y", tag="y")
                            for k in range(2):
                                nc.tensor.matmul(
                                    y_ps[:],
                                    lhsT=WLF[:, k, cc * 128:(cc + 1) * 128],
                                    rhs=xt_res[:, k, sl],
                                    start=(k == 0), stop=False,
                                )
                            for kk in range(8):
                                nc.tensor.matmul(
                                    y_ps[:],
                                    lhsT=w2t[:, kk, cc * 128:(cc + 1) * 128],
                                    rhs=nl[:, kk, 0, :],
                                    start=False, stop=(kk == 7),
                                )
                            if cc == 0:
                                y_sb0 = yp.tile([128, 512], BF16, name="ysb", tag="ysb")
                                nc.scalar.activation(out=y_sb0[:], in_=y_ps[:],
                                                     func=AF.Identity,
                                                     bias=bias[:, 10:11])
                                nc.sync.dma_start(out=yt_d[0, :, sl], in_=y_sb0[:])
                            else:
                                y_sb1 = yp.tile([128, 512], BF16, name="ysb2", tag="ysb2")
                                nc.vector.tensor_scalar(out=y_sb1[:], in0=y_ps[:],
                                                        scalar1=bias[:, 11:12],
                                                        scalar2=None, op0=ALU.add)
                                nc.scalar.dma_start(out=yt_d[1, :, sl], in_=y_sb1[:])
                        break
                    if pending is not None:
                        for g in pending[3]:
                            emit_y_group(pending[0], pending[2], *g)
                    groups = [(cc, nh) for nh in range(nhc) for cc in (0, 1)]
                    pending = (base, nhc, nl, groups)

    _legalize_waits(nc, 1)
    return nc


def _get_program(reps=1):
    global _NC
    if reps != 1:
        return _build_program(reps)
    if _NC is None:
        _NC = _build_program()
    return _NC


def _prep_maps(x, gamma1, w_qkv, w_out, b_out, gamma2, w1, b1, w2, b2):
    f8 = np.float64
    x = np.asarray(x, np.float32)
    g1 = np.asarray(gamma1, f8)
    g2 = np.asarray(gamma2, f8)
    w_qkv = np.asarray(w_qkv, f8)
    w_out = np.asarray(w_out, f8)
    b_out = np.asarray(b_out, f8)
    w1 = np.asarray(w1, f8)
    b1 = np.asarray(b1, f8)
    w2 = np.asarray(w2, f8)
    b2 = np.asarray(b2, f8)

    wq = w_qkv[0:INNER] * g1[None, :]
    wk = w_qkv[INNER:2 * INNER] * g1[None, :]
    wv = w_qkv[2 * INNER:3 * INNER] * g1[None, :]
    w1g = w1 * g2[None, :]
    wlin_m = np.eye(C) + 0.5 * (w2 @ w1g)       # [c', c]
    b2v = b2 + 0.5 * (w2 @ b1)

    def pk(a, kdim):  # [kdim*128, F] -> [128, kdim, F]
        return np.ascontiguousarray(
            a.reshape(kdim, 128, a.shape[-1]).transpose(1, 0, 2)).astype(NP_BF16)

    wkvT = np.concatenate([wk.T, wv.T], axis=1)             # [256, 1024]
    wkv_h = pk(wkvT, 2)
    wq_h = np.ascontiguousarray(
        wq.reshape(H, 64, C).transpose(1, 0, 2)).astype(NP_BF16)
    wo_h = pk(w_out.T.copy(), 4)                            # [512,256]->[128,4,256]
    w1t_h = pk(w1g.T.copy(), 2)                             # [256,1024]->[128,2,1024]
    w2t_h = np.ascontiguousarray(
        w2.T.reshape(8, 128, C).transpose(1, 0, 2)).astype(NP_BF16)
    wlin_h = pk(wlin_m.T.copy(), 2)                         # [256,256]->[128,2,256]
    ident_h = pk(np.eye(C), 2)
    b1f = b_out @ w1g.T + b1                                # [1024]
    b2f = b2v + b_out @ wlin_m.T                            # [256]
    bias_h = np.concatenate([
        b_out.reshape(2, 128).T, b1f.reshape(8, 128).T, b2f.reshape(2, 128).T,
    ], axis=1).astype(np.float32)                           # [128, 12]
    shared = dict(wkv=wkv_h, wq=wq_h, wo=wo_h, w1t=w1t_h, w2t=w2t_h,
                  wlin=wlin_h, ident=ident_h, bias=bias_h)

    in_maps = []
    for b in range(B):
        xb = np.ascontiguousarray(x[b])
        # p-major fp8 layout for the DoubleRow Gram:
        # n = tb*2048 + u*256 + i*128 + p  ->  [p, tb, u*2+i, c]
        xq = np.ascontiguousarray(
            xb.reshape(4, 8, 2, 128, C).transpose(3, 0, 1, 2, 4)
            .reshape(128, 4, 16, C)).astype(NP_F8)
        in_maps.append(dict(xq=xq, xt=np.ascontiguousarray(xb.T).astype(NP_BF16),
                            **shared))
    return in_maps


def kernel(**inputs):
    global LAST_RESULTS
    nc = _get_program()
    in_maps = _prep_maps(**inputs)
    res = run_bass_kernel_spmd(nc, in_maps, list(range(N_CORES)))
    LAST_RESULTS = res
    out = np.empty((B, N, C), np.float32)
    for b in range(B):
        yt = np.asarray(res.results[b]["yt"]).astype(np.float32)   # [2, 128, N]
        out[b] = yt.reshape(C, N).T
    return out
